# revision 19
# baseline (speedup 1.0000x reference)
# Trainium2 Bass kernel for NonLocalBlock (GroupNorm + 1x1-conv self-attention + residual).
#
# Full input x: [4, 256, 64, 64] f32. Output: x + proj(attn(gn(x))), same shape.
#
# Sharding: 8 cores = 4 batches x 2 query-halves. Attention is independent per
# batch; within a batch, softmax rows (queries) split cleanly across 2 cores.
# Each core redundantly computes GroupNorm + K + V^T for its batch (cheap), and
# computes scores/softmax/PV/proj only for its 2048 queries. No collectives.
#
# Per-core program (c = 256 channels as 2 partition-tiles, n = 4096 keys):
#   - GroupNorm stats: bn_stats/bn_aggr per channel, group-combine and
#     broadcast-back via tiny PE matmuls with 0/1 group matrices.
#   - h = x*A + B (bf16), plus the query half from a separate input slice so
#     all access patterns stay static across the SPMD program.
#   - k, q, vT in fp8-e4m3 with the contraction dim stored channel-interleaved
#     ([128, 2, *]), so the attention matmuls run in DoubleRow perf mode
#     (2 fp8 MACs/cell/cycle, K=256 per instruction). The interleave is
#     produced for free: host permutes weight columns; PSUM->SBUF copies land
#     each output-channel half in its pair plane.
#   - scores transposed: sT[j,i] = k^T q; exp on ACT fused with the
#     PSUM->SBUF copy (1/sqrt(c) folded into the activation scale); eT[j,i]
#     is then directly the PV moving operand - no transposes anywhere.
#   - row sums of exp via ones-vector DR matmuls; softmax normalization is a
#     column scaling that commutes through PV and proj, applied in the output
#     stage (reciprocal_approx_fast on a broadcast of the sums).
#   - bv never applied on-chip: softmax rows sum to 1, so wproj@bv folds into
#     bproj on the host. out = x_half + rinv * (wproj @ A_unnorm) + bproj_eff.
#
# Stationary-operand reuse: each k/vT slice serves all 4 query blocks
# back-to-back, so LDWEIGHTS is paid once per 4 matmuls.

import os
import sys

for _p in ("/opt/trn_rl_repo", "/root/.axon_site/_ro/trn_rl_repo"):
    if os.path.isdir(_p) and _p not in sys.path:
        sys.path.insert(0, _p)

import numpy as np
import ml_dtypes

import concourse.bass as bass
import concourse.tile as tile
from concourse import bacc, mybir
from concourse.alu_op_type import AluOpType
from concourse.bass_utils import run_bass_kernel_spmd

from concourse import dve_ops as _dvo
from concourse.dve_spec import Spec as _DveSpec, Src0 as _Src0, C0 as _C0, \
    C1 as _C1, C2 as _C2, sq as _sq, lower as _dve_lower
from concourse.dve_uop import DveOpSpec as _DveOpSpec
from concourse.dve_table_gen import dve_ver_for as _dve_ver_for


def _register_exp_q4():
    # out = (c2 + z*(c1 + z*c0))^4 ~= exp(z*s) when (c0,c1,c2) are the
    # quadratic Taylor of exp(z*s/4): lets the Vector engine carry half the
    # softmax exp load (single fused uop; ACT is otherwise the bottleneck).
    if "EXP_Q4_ANT" in _dvo._SUB_OPCODE_FOR_NAME:
        return next(op for op in _dvo.OPS if op.name == "EXP_Q4_ANT")
    ver = _dve_ver_for("TRN2")
    spec = _DveSpec(
        body=_sq(_sq(_C2 + _Src0 * (_C1 + _Src0 * _C0))),
        reference=lambda in0, in1, c0, c1, c2: (c2 + in0 * (c1 + in0 * c0)) ** 4)
    opcode = max(_dvo._SUB_OPCODE_FOR_NAME.values()) + 1
    sha = _DveOpSpec(name="EXP_Q4_ANT", opcode=opcode,
                     uops=_dve_lower(spec, ver=ver), rd1_en=False).sha(ver)
    op = _dvo.DveOp("EXP_Q4_ANT", spec, subdim=False, uops_sha={ver: sha})
    _dvo.OPS.append(op)
    _dvo.CUSTOM_DVE_SPECS["EXP_Q4_ANT"] = spec
    _dvo._SUB_OPCODE_FOR_NAME["EXP_Q4_ANT"] = opcode
    return op


EXP_Q4 = _register_exp_q4()

F32 = mybir.dt.float32
BF16 = mybir.dt.bfloat16
F8 = mybir.dt.float8e4
AF = mybir.ActivationFunctionType
DR = mybir.MatmulPerfMode.DoubleRow

B = 4
C = 256
N = 4096           # 64*64 spatial positions
NH = N // 2        # queries per core
GROUPS = 32
GSIZE = C // GROUPS  # 8 channels per group
EPS = 1e-6
P = 128
CT = C // P        # 2 channel tiles
JT = N // P        # 32 key tiles (16 DoubleRow pairs)
JP = JT // 2
NB = NH // 512     # 4 query blocks of 512
NCORES = 8
SCALE = float(1.0 / np.sqrt(C))

_cache = {}


def _col(ap_1d, ct):
    # View a [256] DRAM tensor as [256, 1] and take channel-tile ct's [128, 1].
    return ap_1d.ap().rearrange("(a b) -> a b", b=1)[ct * P:(ct + 1) * P, :]


def _build_program():
    nc = bacc.Bacc("TRN2", target_bir_lowering=False, debug=False)

    x_full = nc.dram_tensor("x_full", [C, N], F32, kind="ExternalInput")
    xh = nc.dram_tensor("xh", [C, NH], F32, kind="ExternalInput")
    gnsc = nc.dram_tensor("gnsc", [C], F32, kind="ExternalInput")
    gnbs = nc.dram_tensor("gnbs", [C], F32, kind="ExternalInput")
    g8 = nc.dram_tensor("g8", [P, P // GSIZE], F32, kind="ExternalInput")
    gt01 = nc.dram_tensor("gt01", [P // GSIZE, P], F32, kind="ExternalInput")
    wqT = nc.dram_tensor("wqT", [C, C], BF16, kind="ExternalInput")
    bq = nc.dram_tensor("bq", [C], F32, kind="ExternalInput")
    wkT = nc.dram_tensor("wkT", [C, C], BF16, kind="ExternalInput")
    bk = nc.dram_tensor("bk", [C], F32, kind="ExternalInput")
    wvT = nc.dram_tensor("wvT", [C, C], BF16, kind="ExternalInput")
    wpT = nc.dram_tensor("wpT", [C, C], BF16, kind="ExternalInput")
    bpe = nc.dram_tensor("bpe", [C], F32, kind="ExternalInput")
    out = nc.dram_tensor("out", [C, NH], F32, kind="ExternalOutput")
    rinv_scr = nc.dram_tensor("rinv_scr", [NH], F32)

    with tile.TileContext(nc) as tc:
        _body(tc, x_full, xh, gnsc, gnbs, g8, gt01,
              wqT, bq, wkT, bk, wvT, wpT, bpe, out, rinv_scr)
    nc.compile()
    return nc


def _body(tc, x_full, xh, gnsc, gnbs, g8, gt01,
          wqT, bq, wkT, bk, wvT, wpT, bpe, out, rinv_scr):
    nc = tc.nc
    NG = P // GSIZE  # 16 groups per channel tile

    from contextlib import ExitStack
    with ExitStack() as ctx:
        consts = ctx.enter_context(tc.tile_pool(name="consts", bufs=1))
        px = ctx.enter_context(tc.tile_pool(name="px", bufs=1))
        ph = ctx.enter_context(tc.tile_pool(name="ph", bufs=1))
        pkv = ctx.enter_context(tc.tile_pool(name="pkv", bufs=1))
        pst = ctx.enter_context(tc.tile_pool(name="pst", bufs=4))
        pout = ctx.enter_context(tc.tile_pool(name="pout", bufs=3))
        # PSUM: two 2-bank score/misc slots + two 2-bank PV accumulators = 8
        ps_big = ctx.enter_context(tc.tile_pool(name="ps_big", bufs=2, space="PSUM"))
        ps_sum = ps_big

        # ---- x load first: one 1MB DMA per (ct, half), two queues ----
        x_sb = []
        for ct in range(CT):
            xt = px.tile([P, N], F32, tag=f"x{ct}", name=f"x{ct}")
            for c2 in range(2):
                [nc.sync, nc.scalar, nc.gpsimd, nc.sync][ct * 2 + c2].dma_start(
                    out=xt[:, c2 * 2048:(c2 + 1) * 2048],
                    in_=x_full.ap()[ct * P:(ct + 1) * P, c2 * 2048:(c2 + 1) * 2048])
            x_sb.append(xt)

        # ---- constants (gpsimd queue; keeps x queues clear) ----
        # DR weights need 16B-aligned pair-plane step; pad the ones vector
        ones8_t = consts.tile([P, 2, 16], F8, tag="ones")
        nc.vector.memset(ones8_t, 1.0)
        ones8 = ones8_t[:, :, 0:1]
        g8_sb = consts.tile([P, NG], F32, tag="g8")
        nc.gpsimd.dma_start(out=g8_sb, in_=g8.ap())
        gt01_sb = consts.tile([NG, P], F32, tag="gt01")
        nc.gpsimd.dma_start(out=gt01_sb, in_=gt01.ap())

        w_sb = {}
        for name, h in (("wqT", wqT), ("wkT", wkT), ("wvT", wvT), ("wpT", wpT)):
            for ec in range(CT):
                t = consts.tile([P, C], BF16, tag=f"{name}{ec}")
                nc.gpsimd.dma_start(out=t, in_=h.ap()[ec * P:(ec + 1) * P, :])
                w_sb[(name, ec)] = t

        col_sb = {}
        for name, h in (("gnsc", gnsc), ("gnbs", gnbs), ("bq", bq),
                        ("bk", bk), ("bpe", bpe)):
            for ct in range(CT):
                t = consts.tile([P, 1], F32, tag=f"{name}{ct}")
                nc.gpsimd.dma_start(out=t, in_=_col(h, ct))
                col_sb[(name, ct)] = t

        # ---- GroupNorm stats ----
        ab_cols = []
        for ct in range(CT):
            xt = x_sb[ct]
            stats = pst.tile([P, 8, nc.vector.BN_STATS_DIM], F32, tag="bnst")
            for s in range(8):
                nc.vector.bn_stats(out=stats[:, s, :], in_=xt[:, s * 512:(s + 1) * 512])
            mv = pst.tile([P, nc.vector.BN_AGGR_DIM], F32, tag="bnagg")
            nc.vector.bn_aggr(out=mv, in_=stats)

            # per-channel (mean, E[x^2]) -> per-group via G/8 matmul
            st2 = pst.tile([P, 2], F32, tag="st2")
            nc.vector.tensor_copy(out=st2[:, 0:1], in_=mv[:, 0:1])
            m2 = pst.tile([P, 1], F32, tag="m2")
            nc.vector.tensor_mul(m2, mv[:, 0:1], mv[:, 0:1])
            nc.vector.tensor_add(st2[:, 1:2], m2, mv[:, 1:2])

            gps = ps_big.tile([NG, 2], F32, tag="big")
            nc.tensor.matmul(gps, lhsT=g8_sb, rhs=st2, start=True, stop=True)
            gs = pst.tile([NG, 2], F32, tag="gs")
            nc.vector.tensor_copy(out=gs, in_=gps)

            # var_g = E[x^2]_g - mean_g^2 ; rstd = 1/sqrt(var+eps)
            vg = pst.tile([NG, 1], F32, tag="vg")
            nc.vector.tensor_mul(vg, gs[:, 0:1], gs[:, 0:1])
            nc.vector.tensor_tensor(out=vg, in0=gs[:, 1:2], in1=vg,
                                    op=AluOpType.subtract)
            eps_t = pst.tile([NG, 1], F32, tag="eps")
            nc.vector.memset(eps_t, EPS)
            std = pst.tile([NG, 1], F32, tag="std")
            nc.scalar.activation(out=std, in_=vg, func=AF.Sqrt, bias=eps_t, scale=1.0)
            rstd = pst.tile([NG, 1], F32, tag="rstd")
            nc.vector.reciprocal(out=rstd, in_=std)

            gs2 = pst.tile([NG, 2], F32, tag="gs2")
            nc.vector.tensor_copy(out=gs2[:, 0:1], in_=gs[:, 0:1])
            nc.vector.tensor_copy(out=gs2[:, 1:2], in_=rstd)

            # broadcast (mean_g, rstd_g) back to channels
            bps = ps_big.tile([P, 2], F32, tag="big")
            nc.tensor.matmul(bps, lhsT=gt01_sb, rhs=gs2, start=True, stop=True)
            mr = pst.tile([P, 2], F32, tag="mr")
            nc.vector.tensor_copy(out=mr, in_=bps)

            a_col = pst.tile([P, 1], F32, tag=f"acol{ct}")
            nc.vector.tensor_mul(a_col, mr[:, 1:2], col_sb[("gnsc", ct)])
            b_col = pst.tile([P, 1], F32, tag=f"bcol{ct}")
            nc.vector.tensor_mul(b_col, mr[:, 0:1], a_col)
            nc.vector.tensor_tensor(out=b_col, in0=col_sb[("gnbs", ct)],
                                    in1=b_col, op=AluOpType.subtract)
            ab_cols.append((a_col, b_col))

        # ---- query-half h and q first: q gates the score matmuls, which
        # can then overlap the tail of k/vT production ----
        xh_sb, hh_sb = [], []
        for ct in range(CT):
            xht = px.tile([P, NH], F32, tag=f"x{ct}", name=f"xh{ct}")
            [nc.scalar, nc.sync][ct].dma_start(
                out=xht, in_=xh.ap()[ct * P:(ct + 1) * P, :])
            xh_sb.append(xht)
            a_col, b_col = ab_cols[ct]
            hht = ph.tile([P, NH], BF16, tag=f"hh{ct}", name=f"hh{ct}")
            nc.gpsimd.tensor_scalar(out=hht, in0=xht, scalar1=a_col, scalar2=b_col,
                                    op0=AluOpType.mult, op1=AluOpType.add)
            hh_sb.append(hht)

        q_sb = pkv.tile([P, 2, NH], F8, tag="q")
        for dt in range(CT):
            for icp in range(2):
                ps = ps_big.tile([P, 1024], F32, tag=["big", "pva"][(dt + icp) % 2], name=f"q{dt}_{icp}")
                for ii in range(2):
                    ic = 2 * icp + ii
                    for ec in range(CT):
                        nc.tensor.matmul(
                            ps[:, ii * 512:(ii + 1) * 512],
                            lhsT=w_sb[("wqT", ec)][:, dt * P:(dt + 1) * P],
                            rhs=hh_sb[ec][:, ic * 512:(ic + 1) * 512],
                            start=(ec == 0), stop=(ec == CT - 1))
                nc.scalar.activation(
                    out=q_sb[:, dt, icp * 1024:(icp + 1) * 1024], in_=ps,
                    func=AF.Identity, bias=col_sb[("bq", dt)], scale=1.0)

        # ---- h = x*A+B (chunked so k/vT matmuls start early); k, vT ----
        # k_sb/q_sb/vT_dr are fp8 with channels pair-interleaved for DoubleRow:
        # value (p, q, .) = channel 2p+q (host permuted the weight columns).
        h_sb = [ph.tile([P, N], BF16, tag=f"h{ct}", name=f"h{ct}") for ct in range(CT)]
        k_sb = pkv.tile([P, 2, N], F8, tag="k")
        vT_dr = pkv.tile([P, 2, JP, C], F8, tag="vT")
        for c4 in range(4):
            j0 = c4 * 1024
            for ct in range(CT):
                a_col, b_col = ab_cols[ct]
                nc.gpsimd.tensor_scalar(
                    out=h_sb[ct][:, j0:j0 + 1024], in0=x_sb[ct][:, j0:j0 + 1024],
                    scalar1=a_col, scalar2=b_col,
                    op0=AluOpType.mult, op1=AluOpType.add)
            for dt in range(CT):
                ps = ps_big.tile([P, 1024], F32, tag=["big", "pva"][(c4 + dt) % 2], name=f"k{c4}_{dt}")
                for jj in range(2):
                    jc = 2 * c4 + jj
                    for ec in range(CT):
                        nc.tensor.matmul(
                            ps[:, jj * 512:(jj + 1) * 512],
                            lhsT=w_sb[("wkT", ec)][:, dt * P:(dt + 1) * P],
                            rhs=h_sb[ec][:, jc * 512:(jc + 1) * 512],
                            start=(ec == 0), stop=(ec == CT - 1))
                nc.scalar.activation(
                    out=k_sb[:, dt, j0:j0 + 1024], in_=ps,
                    func=AF.Identity, bias=col_sb[("bk", dt)], scale=1.0)
            for t in (2 * c4, 2 * c4 + 1):
                # four jt per psum tile, quarters ordered (q, jtp) so one copy
                # lands them all in vT_dr[:, :, 2t:2t+2, :]
                ps = ps_big.tile([P, 4, C], F32, tag=["big", "pva"][t % 2],
                                 name=f"v{t}")
                for jj in range(4):
                    jt = 4 * t + jj
                    quarter = (jt % 2) * 2 + (jt // 2) % 2
                    for ec in range(CT):
                        nc.tensor.matmul(
                            ps[:, quarter, :],
                            lhsT=h_sb[ec][:, jt * P:(jt + 1) * P],
                            rhs=w_sb[("wvT", ec)],
                            start=(ec == 0), stop=(ec == CT - 1))
                nc.scalar.activation(out=vT_dr[:, :, 2 * t:2 * t + 2, :],
                                     in_=ps, func=AF.Copy)

        # ---- attention: one pass over the 32 key tiles for all 2048
        # queries. Per key tile: 4 DR score matmuls; exp of the first query
        # half on ACT, of the second half on the Vector engine (fused q^4
        # polynomial - splitting exp across engines is what keeps PE fed).
        # PV for channel-tile 0 rides along; sums/PV-ct1/proj follow.
        # eT[p, jtp, q, i] = exp(s[j=(2*jtp+q)*128+p, i]/16)  (fp8)
        eT = pkv.tile([P, JP, 2, NH], F8, tag="eT")
        A_sb = [pkv.tile([P, NH], BF16, tag=f"A{ct}", name=f"A{ct}")
                for ct in range(CT)]
        rinvb = pkv.tile([P, NH], F32, tag="rinvb")
        EC1 = SCALE / 4.0
        EC0 = SCALE * SCALE / 32.0

        # scores: four rotating psum slots (both tag groups) so the two exp
        # engines pipeline freely; per jt, half0 exps on ACT, half1 on DVE
        for jt in range(JT):
            kw = k_sb[:, :, jt * P:(jt + 1) * P]
            for half in range(2):
                ps = ps_big.tile([P, 1024], F32,
                                 tag=["big", "pva"][jt % 2],
                                 name=f"sc{jt}_{half}")
                for ii in range(2):
                    ib = 2 * half + ii
                    nc.tensor.matmul(
                        ps[:, ii * 512:(ii + 1) * 512], lhsT=kw,
                        rhs=q_sb[:, :, ib * 512:(ib + 1) * 512],
                        start=True, stop=True, perf_mode=DR)
                dst = eT[:, jt // 2, jt % 2, half * 1024:(half + 1) * 1024]
                if half == 0:
                    nc.scalar.activation(out=dst, in_=ps, func=AF.Exp,
                                         scale=SCALE)
                else:
                    nc.vector._custom_dve(EXP_Q4, out=dst, in0=ps,
                                          s0=EC0, s1=EC1, imm2=1.0)

        # row sums: 4 accumulators spread over both tag groups
        pssums = [ps_big.tile([1, 512], F32, tag=["big", "pva"][ib % 2],
                              name=f"sm{ib}") for ib in range(NB)]
        for jtp in range(JP):
            for ib in range(NB):
                nc.tensor.matmul(pssums[ib], lhsT=ones8,
                                 rhs=eT[:, jtp, :, ib * 512:(ib + 1) * 512],
                                 start=(jtp == 0), stop=(jtp == JP - 1),
                                 perf_mode=DR)
        for ib in range(NB):
            i0 = ib * 512
            srow = pst.tile([1, 512], F32, tag="srow")
            nc.vector.tensor_copy(out=srow, in_=pssums[ib])
            nc.sync.dma_start(
                out=rinv_scr.ap().rearrange("(a b) -> a b", a=1)[:, i0:i0 + 512],
                in_=srow)
            rsc = rinv_scr.ap()[i0:i0 + 512]
            sb = pout.tile([P, 512], F32, tag="sb")
            nc.gpsimd.dma_start(
                out=sb,
                in_=bass.AP(tensor=rsc.tensor, offset=rsc.offset,
                            ap=[[0, P]] + [list(d) for d in rsc.ap]))
            nc.vector.reciprocal_approx_fast(out=rinvb[:, i0:i0 + 512],
                                             in_=sb)

        # PV: both channel tiles accumulate concurrently (8 banks), each
        # vT slice stationary across 4 matmuls
        psas = {(ct, h): ps_big.tile([P, 1024], F32,
                                     tag=["big", "pva"][ct],
                                     name=f"pv{ct}_{h}")
                for ct in range(CT) for h in range(2)}
        for jtp in range(JP):
            for ct in range(CT):
                vw = vT_dr[:, :, jtp, ct * P:(ct + 1) * P]
                for half in range(2):
                    for ii in range(2):
                        ib = 2 * half + ii
                        nc.tensor.matmul(
                            psas[(ct, half)][:, ii * 512:(ii + 1) * 512],
                            lhsT=vw,
                            rhs=eT[:, jtp, :, ib * 512:(ib + 1) * 512],
                            start=(jtp == 0), stop=(jtp == JP - 1),
                            perf_mode=DR)
        for ct in range(CT):
            for half in range(2):
                nc.scalar.activation(
                    out=A_sb[ct][:, half * 1024:(half + 1) * 1024],
                    in_=psas[(ct, half)], func=AF.Copy)

        # ---- output projection + normalization + bias + residual ----
        for dt in range(CT):
            for icp in range(2):
                i0 = icp * 1024
                ps = ps_big.tile([P, 1024], F32, tag=["big", "pva"][(dt + icp) % 2],
                                 name=f"pj{dt}_{icp}")
                for ii in range(2):
                    ic = 2 * icp + ii
                    for cc in range(CT):
                        nc.tensor.matmul(
                            ps[:, ii * 512:(ii + 1) * 512],
                            lhsT=w_sb[("wpT", cc)][:, dt * P:(dt + 1) * P],
                            rhs=A_sb[cc][:, ic * 512:(ic + 1) * 512],
                            start=(cc == 0), stop=(cc == CT - 1))
                ot = pout.tile([P, 1024], F32, tag="ot")
                nc.vector.tensor_mul(ot, ps, rinvb[:, i0:i0 + 1024])
                nc.vector.scalar_tensor_tensor(
                    out=ot, in0=ot, scalar=col_sb[("bpe", dt)],
                    in1=xh_sb[dt][:, i0:i0 + 1024],
                    op0=AluOpType.add, op1=AluOpType.add)
                nc.sync.dma_start(
                    out=out.ap()[dt * P:(dt + 1) * P, i0:i0 + 1024],
                    in_=ot)


def _get_program():
    if "nc" not in _cache:
        _cache["nc"] = _build_program()
    return _cache["nc"]


def kernel(x, gn_scale, gn_bias, wq, bq, wk, bk, wv, bv, wproj, bproj):
    x = np.asarray(x, dtype=np.float32)
    b, c, hh, ww = x.shape
    assert (b, c, hh * ww) == (B, C, N)
    xf = np.ascontiguousarray(x.reshape(B, C, N))

    bf = ml_dtypes.bfloat16
    # Channel-pair interleave permutation for DoubleRow: even channels then odd.
    perm = np.concatenate([np.arange(0, C, 2), np.arange(1, C, 2)])
    wqT_s = np.ascontiguousarray(np.asarray(wq, np.float32).T[:, perm]).astype(bf)
    bq_s = np.ascontiguousarray(np.asarray(bq, np.float32)[perm])
    wkT = np.ascontiguousarray(np.asarray(wk, np.float32).T[:, perm]).astype(bf)
    bk_s = np.ascontiguousarray(np.asarray(bk, np.float32)[perm])
    wvT = np.ascontiguousarray(np.asarray(wv, np.float32).T[:, perm]).astype(bf)
    wpT = np.ascontiguousarray(np.asarray(wproj, np.float32).T[perm, :]).astype(bf)
    # softmax rows sum to 1 => v-bias contributes wproj@bv, constant per channel
    bpe = (np.asarray(bproj, np.float64)
           + np.asarray(wproj, np.float64) @ np.asarray(bv, np.float64)
           ).astype(np.float32)

    g8 = np.zeros((P, P // GSIZE), np.float32)
    gt01 = np.zeros((P // GSIZE, P), np.float32)
    for ch in range(P):
        g8[ch, ch // GSIZE] = 1.0 / GSIZE   # yields per-group means directly
        gt01[ch // GSIZE, ch] = 1.0

    common = dict(gnsc=np.asarray(gn_scale, np.float32),
                  gnbs=np.asarray(gn_bias, np.float32),
                  g8=g8, gt01=gt01,
                  wqT=wqT_s, bq=bq_s, wkT=wkT, bk=bk_s,
                  wvT=wvT, wpT=wpT, bpe=bpe)

    in_maps = []
    for core in range(NCORES):
        bi, half = core // 2, core % 2
        in_maps.append(dict(
            x_full=np.ascontiguousarray(xf[bi]),
            xh=np.ascontiguousarray(xf[bi][:, half * NH:(half + 1) * NH]),
            **common))

    nc = _get_program()
    trace = bool(os.environ.get("BASS_KERNEL_TRACE"))
    res = run_bass_kernel_spmd(nc, in_maps, core_ids=list(range(NCORES)),
                               trace=trace)
    _cache["last_results"] = res

    full = np.empty((B, C, N), np.float32)
    for core in range(NCORES):
        bi, half = core // 2, core % 2
        full[bi][:, half * NH:(half + 1) * NH] = res.results[core]["out"]
    return full.reshape(B, C, hh, ww)


# revision 20
# speedup vs baseline: 1.0451x; 1.0451x over previous
# Trainium2 Bass kernel for NonLocalBlock (GroupNorm + 1x1-conv self-attention + residual).
#
# Full input x: [4, 256, 64, 64] f32. Output: x + proj(attn(gn(x))), same shape.
#
# Sharding: 8 cores = 4 batches x 2 query-halves. Attention is independent per
# batch; within a batch, softmax rows (queries) split cleanly across 2 cores.
# Each core redundantly computes GroupNorm + K + V^T for its batch (cheap), and
# computes scores/softmax/PV/proj only for its 2048 queries. No collectives.
#
# Per-core program (c = 256 channels as 2 partition-tiles, n = 4096 keys):
#   - GroupNorm stats: bn_stats/bn_aggr per channel, group-combine and
#     broadcast-back via tiny PE matmuls with 0/1 group matrices.
#   - h = x*A + B (bf16), plus the query half from a separate input slice so
#     all access patterns stay static across the SPMD program.
#   - k, q, vT in fp8-e4m3 with the contraction dim stored channel-interleaved
#     ([128, 2, *]), so the attention matmuls run in DoubleRow perf mode
#     (2 fp8 MACs/cell/cycle, K=256 per instruction). The interleave is
#     produced for free: host permutes weight columns; PSUM->SBUF copies land
#     each output-channel half in its pair plane.
#   - scores transposed: sT[j,i] = k^T q; exp on ACT fused with the
#     PSUM->SBUF copy (1/sqrt(c) folded into the activation scale); eT[j,i]
#     is then directly the PV moving operand - no transposes anywhere.
#   - row sums of exp via ones-vector DR matmuls; softmax normalization is a
#     column scaling that commutes through PV and proj, applied in the output
#     stage (reciprocal_approx_fast on a broadcast of the sums).
#   - bv never applied on-chip: softmax rows sum to 1, so wproj@bv folds into
#     bproj on the host. out = x_half + rinv * (wproj @ A_unnorm) + bproj_eff.
#
# Stationary-operand reuse: each k/vT slice serves all 4 query blocks
# back-to-back, so LDWEIGHTS is paid once per 4 matmuls.

import os
import sys

for _p in ("/opt/trn_rl_repo", "/root/.axon_site/_ro/trn_rl_repo"):
    if os.path.isdir(_p) and _p not in sys.path:
        sys.path.insert(0, _p)

import numpy as np
import ml_dtypes

import concourse.bass as bass
import concourse.tile as tile
from concourse import bacc, mybir
from concourse.alu_op_type import AluOpType
from concourse.bass_utils import run_bass_kernel_spmd

from concourse import dve_ops as _dvo
from concourse.dve_spec import Spec as _DveSpec, Src0 as _Src0, C0 as _C0, \
    C1 as _C1, C2 as _C2, sq as _sq, lower as _dve_lower
from concourse.dve_uop import DveOpSpec as _DveOpSpec
from concourse.dve_table_gen import dve_ver_for as _dve_ver_for


def _register_exp_q4():
    # out = (c2 + z*(c1 + z*c0))^4 ~= exp(z*s) when (c0,c1,c2) are the
    # quadratic Taylor of exp(z*s/4): lets the Vector engine carry half the
    # softmax exp load (single fused uop; ACT is otherwise the bottleneck).
    if "EXP_Q4_ANT" in _dvo._SUB_OPCODE_FOR_NAME:
        return next(op for op in _dvo.OPS if op.name == "EXP_Q4_ANT")
    ver = _dve_ver_for("TRN2")
    spec = _DveSpec(
        body=_sq(_sq(_C2 + _Src0 * (_C1 + _Src0 * _C0))),
        reference=lambda in0, in1, c0, c1, c2: (c2 + in0 * (c1 + in0 * c0)) ** 4)
    opcode = max(_dvo._SUB_OPCODE_FOR_NAME.values()) + 1
    sha = _DveOpSpec(name="EXP_Q4_ANT", opcode=opcode,
                     uops=_dve_lower(spec, ver=ver), rd1_en=False).sha(ver)
    op = _dvo.DveOp("EXP_Q4_ANT", spec, subdim=False, uops_sha={ver: sha})
    _dvo.OPS.append(op)
    _dvo.CUSTOM_DVE_SPECS["EXP_Q4_ANT"] = spec
    _dvo._SUB_OPCODE_FOR_NAME["EXP_Q4_ANT"] = opcode
    return op


EXP_Q4 = _register_exp_q4()

F32 = mybir.dt.float32
BF16 = mybir.dt.bfloat16
F8 = mybir.dt.float8e4
AF = mybir.ActivationFunctionType
DR = mybir.MatmulPerfMode.DoubleRow

B = 4
C = 256
N = 4096           # 64*64 spatial positions
NH = N // 2        # queries per core
GROUPS = 32
GSIZE = C // GROUPS  # 8 channels per group
EPS = 1e-6
P = 128
CT = C // P        # 2 channel tiles
JT = N // P        # 32 key tiles (16 DoubleRow pairs)
JP = JT // 2
NB = NH // 512     # 4 query blocks of 512
NCORES = 8
SCALE = float(1.0 / np.sqrt(C))

_cache = {}


def _col(ap_1d, ct):
    # View a [256] DRAM tensor as [256, 1] and take channel-tile ct's [128, 1].
    return ap_1d.ap().rearrange("(a b) -> a b", b=1)[ct * P:(ct + 1) * P, :]


def _build_program():
    nc = bacc.Bacc("TRN2", target_bir_lowering=False, debug=False)

    x_full = nc.dram_tensor("x_full", [C, N], F32, kind="ExternalInput")
    xh = nc.dram_tensor("xh", [C, NH], F32, kind="ExternalInput")
    gnsc = nc.dram_tensor("gnsc", [C], F32, kind="ExternalInput")
    gnbs = nc.dram_tensor("gnbs", [C], F32, kind="ExternalInput")
    g8 = nc.dram_tensor("g8", [P, P // GSIZE], F32, kind="ExternalInput")
    gt01 = nc.dram_tensor("gt01", [P // GSIZE, P], F32, kind="ExternalInput")
    wqT = nc.dram_tensor("wqT", [C, C], BF16, kind="ExternalInput")
    bq = nc.dram_tensor("bq", [C], F32, kind="ExternalInput")
    wkT = nc.dram_tensor("wkT", [C, C], BF16, kind="ExternalInput")
    bk = nc.dram_tensor("bk", [C], F32, kind="ExternalInput")
    wvT = nc.dram_tensor("wvT", [C, C], BF16, kind="ExternalInput")
    wpT = nc.dram_tensor("wpT", [C, C], BF16, kind="ExternalInput")
    bpe = nc.dram_tensor("bpe", [C], F32, kind="ExternalInput")
    out = nc.dram_tensor("out", [C, NH], F32, kind="ExternalOutput")
    rinv_scr = nc.dram_tensor("rinv_scr", [NH], F32)

    with tile.TileContext(nc) as tc:
        _body(tc, x_full, xh, gnsc, gnbs, g8, gt01,
              wqT, bq, wkT, bk, wvT, wpT, bpe, out, rinv_scr)
    nc.compile()
    return nc


def _body(tc, x_full, xh, gnsc, gnbs, g8, gt01,
          wqT, bq, wkT, bk, wvT, wpT, bpe, out, rinv_scr):
    nc = tc.nc
    NG = P // GSIZE  # 16 groups per channel tile

    from contextlib import ExitStack
    with ExitStack() as ctx:
        consts = ctx.enter_context(tc.tile_pool(name="consts", bufs=1))
        px = ctx.enter_context(tc.tile_pool(name="px", bufs=1))
        ph = ctx.enter_context(tc.tile_pool(name="ph", bufs=1))
        pkv = ctx.enter_context(tc.tile_pool(name="pkv", bufs=1))
        pst = ctx.enter_context(tc.tile_pool(name="pst", bufs=4))
        pout = ctx.enter_context(tc.tile_pool(name="pout", bufs=3))
        # PSUM: two 2-bank score/misc slots + two 2-bank PV accumulators = 8
        ps_big = ctx.enter_context(tc.tile_pool(name="ps_big", bufs=2, space="PSUM"))
        ps_sum = ps_big

        # ---- x load first: one 1MB DMA per (ct, half), two queues ----
        x_sb = []
        for ct in range(CT):
            xt = px.tile([P, N], F32, tag=f"x{ct}", name=f"x{ct}")
            for c2 in range(2):
                [nc.sync, nc.scalar, nc.gpsimd, nc.sync][ct * 2 + c2].dma_start(
                    out=xt[:, c2 * 2048:(c2 + 1) * 2048],
                    in_=x_full.ap()[ct * P:(ct + 1) * P, c2 * 2048:(c2 + 1) * 2048])
            x_sb.append(xt)

        # ---- constants (gpsimd queue; keeps x queues clear) ----
        # DR weights need 16B-aligned pair-plane step; pad the ones vector
        ones8_t = consts.tile([P, 2, 16], F8, tag="ones")
        nc.vector.memset(ones8_t, 1.0)
        ones8 = ones8_t[:, :, 0:1]
        g8_sb = consts.tile([P, NG], F32, tag="g8")
        nc.sync.dma_start(out=g8_sb, in_=g8.ap())
        gt01_sb = consts.tile([NG, P], F32, tag="gt01")
        nc.scalar.dma_start(out=gt01_sb, in_=gt01.ap())

        w_sb = {}
        for name, h in (("wqT", wqT), ("wkT", wkT), ("wvT", wvT), ("wpT", wpT)):
            for ec in range(CT):
                t = consts.tile([P, C], BF16, tag=f"{name}{ec}")
                [nc.sync, nc.scalar][ec].dma_start(
                    out=t, in_=h.ap()[ec * P:(ec + 1) * P, :])
                w_sb[(name, ec)] = t

        col_sb = {}
        for name, h in (("gnsc", gnsc), ("gnbs", gnbs), ("bq", bq),
                        ("bk", bk), ("bpe", bpe)):
            for ct in range(CT):
                t = consts.tile([P, 1], F32, tag=f"{name}{ct}")
                [nc.sync, nc.scalar][ct].dma_start(out=t, in_=_col(h, ct))
                col_sb[(name, ct)] = t

        # ---- GroupNorm stats ----
        ab_cols = []
        for ct in range(CT):
            xt = x_sb[ct]
            stats = pst.tile([P, 8, nc.vector.BN_STATS_DIM], F32, tag="bnst")
            for s in range(8):
                nc.vector.bn_stats(out=stats[:, s, :], in_=xt[:, s * 512:(s + 1) * 512])
            mv = pst.tile([P, nc.vector.BN_AGGR_DIM], F32, tag="bnagg")
            nc.vector.bn_aggr(out=mv, in_=stats)

            # per-channel (mean, E[x^2]) -> per-group via G/8 matmul
            st2 = pst.tile([P, 2], F32, tag="st2")
            nc.vector.tensor_copy(out=st2[:, 0:1], in_=mv[:, 0:1])
            m2 = pst.tile([P, 1], F32, tag="m2")
            nc.vector.tensor_mul(m2, mv[:, 0:1], mv[:, 0:1])
            nc.vector.tensor_add(st2[:, 1:2], m2, mv[:, 1:2])

            gps = ps_big.tile([NG, 2], F32, tag="big")
            nc.tensor.matmul(gps, lhsT=g8_sb, rhs=st2, start=True, stop=True)
            gs = pst.tile([NG, 2], F32, tag="gs")
            nc.vector.tensor_copy(out=gs, in_=gps)

            # var_g = E[x^2]_g - mean_g^2 ; rstd = 1/sqrt(var+eps)
            vg = pst.tile([NG, 1], F32, tag="vg")
            nc.vector.tensor_mul(vg, gs[:, 0:1], gs[:, 0:1])
            nc.vector.tensor_tensor(out=vg, in0=gs[:, 1:2], in1=vg,
                                    op=AluOpType.subtract)
            eps_t = pst.tile([NG, 1], F32, tag="eps")
            nc.vector.memset(eps_t, EPS)
            std = pst.tile([NG, 1], F32, tag="std")
            nc.scalar.activation(out=std, in_=vg, func=AF.Sqrt, bias=eps_t, scale=1.0)
            rstd = pst.tile([NG, 1], F32, tag="rstd")
            nc.vector.reciprocal(out=rstd, in_=std)

            gs2 = pst.tile([NG, 2], F32, tag="gs2")
            nc.vector.tensor_copy(out=gs2[:, 0:1], in_=gs[:, 0:1])
            nc.vector.tensor_copy(out=gs2[:, 1:2], in_=rstd)

            # broadcast (mean_g, rstd_g) back to channels
            bps = ps_big.tile([P, 2], F32, tag="big")
            nc.tensor.matmul(bps, lhsT=gt01_sb, rhs=gs2, start=True, stop=True)
            mr = pst.tile([P, 2], F32, tag="mr")
            nc.vector.tensor_copy(out=mr, in_=bps)

            a_col = pst.tile([P, 1], F32, tag=f"acol{ct}")
            nc.vector.tensor_mul(a_col, mr[:, 1:2], col_sb[("gnsc", ct)])
            b_col = pst.tile([P, 1], F32, tag=f"bcol{ct}")
            nc.vector.tensor_mul(b_col, mr[:, 0:1], a_col)
            nc.vector.tensor_tensor(out=b_col, in0=col_sb[("gnbs", ct)],
                                    in1=b_col, op=AluOpType.subtract)
            ab_cols.append((a_col, b_col))

        # ---- h = x*A+B (chunked so k/vT matmuls start early); k, vT ----
        # k_sb/q_sb/vT_dr are fp8 with channels pair-interleaved for DoubleRow:
        # value (p, q, .) = channel 2p+q (host permuted the weight columns).
        h_sb = [ph.tile([P, N], BF16, tag=f"h{ct}", name=f"h{ct}") for ct in range(CT)]
        k_sb = pkv.tile([P, 2, N], F8, tag="k")
        vT_dr = pkv.tile([P, 2, JP, C], F8, tag="vT")
        for c4 in range(4):
            j0 = c4 * 1024
            for ct in range(CT):
                a_col, b_col = ab_cols[ct]
                nc.gpsimd.tensor_scalar(
                    out=h_sb[ct][:, j0:j0 + 1024], in0=x_sb[ct][:, j0:j0 + 1024],
                    scalar1=a_col, scalar2=b_col,
                    op0=AluOpType.mult, op1=AluOpType.add)
            for dt in range(CT):
                ps = ps_big.tile([P, 1024], F32, tag=["big", "pva"][(c4 + dt) % 2], name=f"k{c4}_{dt}")
                for jj in range(2):
                    jc = 2 * c4 + jj
                    for ec in range(CT):
                        nc.tensor.matmul(
                            ps[:, jj * 512:(jj + 1) * 512],
                            lhsT=w_sb[("wkT", ec)][:, dt * P:(dt + 1) * P],
                            rhs=h_sb[ec][:, jc * 512:(jc + 1) * 512],
                            start=(ec == 0), stop=(ec == CT - 1))
                nc.scalar.activation(
                    out=k_sb[:, dt, j0:j0 + 1024], in_=ps,
                    func=AF.Identity, bias=col_sb[("bk", dt)], scale=1.0)
            for t in (2 * c4, 2 * c4 + 1):
                # four jt per psum tile, quarters ordered (q, jtp) so one copy
                # lands them all in vT_dr[:, :, 2t:2t+2, :]
                ps = ps_big.tile([P, 4, C], F32, tag=["big", "pva"][t % 2],
                                 name=f"v{t}")
                for jj in range(4):
                    jt = 4 * t + jj
                    quarter = (jt % 2) * 2 + (jt // 2) % 2
                    for ec in range(CT):
                        nc.tensor.matmul(
                            ps[:, quarter, :],
                            lhsT=h_sb[ec][:, jt * P:(jt + 1) * P],
                            rhs=w_sb[("wvT", ec)],
                            start=(ec == 0), stop=(ec == CT - 1))
                nc.scalar.activation(out=vT_dr[:, :, 2 * t:2 * t + 2, :],
                                     in_=ps, func=AF.Copy)

        # ---- query-half h, q ----
        xh_sb, hh_sb = [], []
        for ct in range(CT):
            xht = px.tile([P, NH], F32, tag=f"x{ct}", name=f"xh{ct}")
            [nc.scalar, nc.sync][ct].dma_start(
                out=xht, in_=xh.ap()[ct * P:(ct + 1) * P, :])
            xh_sb.append(xht)
            a_col, b_col = ab_cols[ct]
            hht = ph.tile([P, NH], BF16, tag=f"hh{ct}", name=f"hh{ct}")
            nc.gpsimd.tensor_scalar(out=hht, in0=xht, scalar1=a_col, scalar2=b_col,
                                    op0=AluOpType.mult, op1=AluOpType.add)
            hh_sb.append(hht)

        q_sb = pkv.tile([P, 2, NH], F8, tag="q")
        for dt in range(CT):
            for icp in range(2):
                ps = ps_big.tile([P, 1024], F32, tag=["big", "pva"][(dt + icp) % 2], name=f"q{dt}_{icp}")
                for ii in range(2):
                    ic = 2 * icp + ii
                    for ec in range(CT):
                        nc.tensor.matmul(
                            ps[:, ii * 512:(ii + 1) * 512],
                            lhsT=w_sb[("wqT", ec)][:, dt * P:(dt + 1) * P],
                            rhs=hh_sb[ec][:, ic * 512:(ic + 1) * 512],
                            start=(ec == 0), stop=(ec == CT - 1))
                nc.scalar.activation(
                    out=q_sb[:, dt, icp * 1024:(icp + 1) * 1024], in_=ps,
                    func=AF.Identity, bias=col_sb[("bq", dt)], scale=1.0)

        # ---- attention: one pass over the 32 key tiles for all 2048
        # queries. Per key tile: 4 DR score matmuls; exp of the first query
        # half on ACT, of the second half on the Vector engine (fused q^4
        # polynomial - splitting exp across engines is what keeps PE fed).
        # PV for channel-tile 0 rides along; sums/PV-ct1/proj follow.
        # eT[p, jtp, q, i] = exp(s[j=(2*jtp+q)*128+p, i]/16)  (fp8)
        eT = pkv.tile([P, JP, 2, NH], F8, tag="eT")
        A_sb = [pkv.tile([P, NH], BF16, tag=f"A{ct}", name=f"A{ct}")
                for ct in range(CT)]
        rinvb = pkv.tile([P, NH], F32, tag="rinvb")
        EC1 = SCALE / 4.0
        EC0 = SCALE * SCALE / 32.0

        # scores: four rotating psum slots (both tag groups) so the two exp
        # engines pipeline freely; per jt, half0 exps on ACT, half1 on DVE
        for jt in range(JT):
            kw = k_sb[:, :, jt * P:(jt + 1) * P]
            for half in range(2):
                ps = ps_big.tile([P, 1024], F32,
                                 tag=["big", "pva"][jt % 2],
                                 name=f"sc{jt}_{half}")
                for ii in range(2):
                    ib = 2 * half + ii
                    nc.tensor.matmul(
                        ps[:, ii * 512:(ii + 1) * 512], lhsT=kw,
                        rhs=q_sb[:, :, ib * 512:(ib + 1) * 512],
                        start=True, stop=True, perf_mode=DR)
                dst = eT[:, jt // 2, jt % 2, half * 1024:(half + 1) * 1024]
                if half == 0:
                    nc.scalar.activation(out=dst, in_=ps, func=AF.Exp,
                                         scale=SCALE)
                else:
                    nc.vector._custom_dve(EXP_Q4, out=dst, in0=ps,
                                          s0=EC0, s1=EC1, imm2=1.0)

        # row sums: 4 accumulators spread over both tag groups
        pssums = [ps_big.tile([1, 512], F32, tag=["big", "pva"][ib % 2],
                              name=f"sm{ib}") for ib in range(NB)]
        for jtp in range(JP):
            for ib in range(NB):
                nc.tensor.matmul(pssums[ib], lhsT=ones8,
                                 rhs=eT[:, jtp, :, ib * 512:(ib + 1) * 512],
                                 start=(jtp == 0), stop=(jtp == JP - 1),
                                 perf_mode=DR)
        for ib in range(NB):
            i0 = ib * 512
            srow = pst.tile([1, 512], F32, tag="srow")
            nc.vector.tensor_copy(out=srow, in_=pssums[ib])
            nc.sync.dma_start(
                out=rinv_scr.ap().rearrange("(a b) -> a b", a=1)[:, i0:i0 + 512],
                in_=srow)
            rsc = rinv_scr.ap()[i0:i0 + 512]
            sb = pout.tile([P, 512], F32, tag="sb")
            nc.gpsimd.dma_start(
                out=sb,
                in_=bass.AP(tensor=rsc.tensor, offset=rsc.offset,
                            ap=[[0, P]] + [list(d) for d in rsc.ap]))
            nc.vector.reciprocal_approx_fast(out=rinvb[:, i0:i0 + 512],
                                             in_=sb)

        # PV: both channel tiles accumulate concurrently (8 banks), each
        # vT slice stationary across 4 matmuls
        psas = {(ct, h): ps_big.tile([P, 1024], F32,
                                     tag=["big", "pva"][ct],
                                     name=f"pv{ct}_{h}")
                for ct in range(CT) for h in range(2)}
        for jtp in range(JP):
            for ct in range(CT):
                vw = vT_dr[:, :, jtp, ct * P:(ct + 1) * P]
                for half in range(2):
                    for ii in range(2):
                        ib = 2 * half + ii
                        nc.tensor.matmul(
                            psas[(ct, half)][:, ii * 512:(ii + 1) * 512],
                            lhsT=vw,
                            rhs=eT[:, jtp, :, ib * 512:(ib + 1) * 512],
                            start=(jtp == 0), stop=(jtp == JP - 1),
                            perf_mode=DR)
        for ct in range(CT):
            for half in range(2):
                nc.scalar.activation(
                    out=A_sb[ct][:, half * 1024:(half + 1) * 1024],
                    in_=psas[(ct, half)], func=AF.Copy)

        # ---- output projection + normalization + bias + residual ----
        for dt in range(CT):
            for icp in range(2):
                i0 = icp * 1024
                ps = ps_big.tile([P, 1024], F32, tag=["big", "pva"][(dt + icp) % 2],
                                 name=f"pj{dt}_{icp}")
                for ii in range(2):
                    ic = 2 * icp + ii
                    for cc in range(CT):
                        nc.tensor.matmul(
                            ps[:, ii * 512:(ii + 1) * 512],
                            lhsT=w_sb[("wpT", cc)][:, dt * P:(dt + 1) * P],
                            rhs=A_sb[cc][:, ic * 512:(ic + 1) * 512],
                            start=(cc == 0), stop=(cc == CT - 1))
                ot = pout.tile([P, 1024], F32, tag="ot")
                nc.vector.tensor_mul(ot, ps, rinvb[:, i0:i0 + 1024])
                nc.vector.scalar_tensor_tensor(
                    out=ot, in0=ot, scalar=col_sb[("bpe", dt)],
                    in1=xh_sb[dt][:, i0:i0 + 1024],
                    op0=AluOpType.add, op1=AluOpType.add)
                nc.sync.dma_start(
                    out=out.ap()[dt * P:(dt + 1) * P, i0:i0 + 1024],
                    in_=ot)


def _get_program():
    if "nc" not in _cache:
        _cache["nc"] = _build_program()
    return _cache["nc"]


def kernel(x, gn_scale, gn_bias, wq, bq, wk, bk, wv, bv, wproj, bproj):
    x = np.asarray(x, dtype=np.float32)
    b, c, hh, ww = x.shape
    assert (b, c, hh * ww) == (B, C, N)
    xf = np.ascontiguousarray(x.reshape(B, C, N))

    bf = ml_dtypes.bfloat16
    # Channel-pair interleave permutation for DoubleRow: even channels then odd.
    perm = np.concatenate([np.arange(0, C, 2), np.arange(1, C, 2)])
    wqT_s = np.ascontiguousarray(np.asarray(wq, np.float32).T[:, perm]).astype(bf)
    bq_s = np.ascontiguousarray(np.asarray(bq, np.float32)[perm])
    wkT = np.ascontiguousarray(np.asarray(wk, np.float32).T[:, perm]).astype(bf)
    bk_s = np.ascontiguousarray(np.asarray(bk, np.float32)[perm])
    wvT = np.ascontiguousarray(np.asarray(wv, np.float32).T[:, perm]).astype(bf)
    wpT = np.ascontiguousarray(np.asarray(wproj, np.float32).T[perm, :]).astype(bf)
    # softmax rows sum to 1 => v-bias contributes wproj@bv, constant per channel
    bpe = (np.asarray(bproj, np.float64)
           + np.asarray(wproj, np.float64) @ np.asarray(bv, np.float64)
           ).astype(np.float32)

    g8 = np.zeros((P, P // GSIZE), np.float32)
    gt01 = np.zeros((P // GSIZE, P), np.float32)
    for ch in range(P):
        g8[ch, ch // GSIZE] = 1.0 / GSIZE   # yields per-group means directly
        gt01[ch // GSIZE, ch] = 1.0

    common = dict(gnsc=np.asarray(gn_scale, np.float32),
                  gnbs=np.asarray(gn_bias, np.float32),
                  g8=g8, gt01=gt01,
                  wqT=wqT_s, bq=bq_s, wkT=wkT, bk=bk_s,
                  wvT=wvT, wpT=wpT, bpe=bpe)

    in_maps = []
    for core in range(NCORES):
        bi, half = core // 2, core % 2
        in_maps.append(dict(
            x_full=np.ascontiguousarray(xf[bi]),
            xh=np.ascontiguousarray(xf[bi][:, half * NH:(half + 1) * NH]),
            **common))

    nc = _get_program()
    trace = bool(os.environ.get("BASS_KERNEL_TRACE"))
    res = run_bass_kernel_spmd(nc, in_maps, core_ids=list(range(NCORES)),
                               trace=trace)
    _cache["last_results"] = res

    full = np.empty((B, C, N), np.float32)
    for core in range(NCORES):
        bi, half = core // 2, core % 2
        full[bi][:, half * NH:(half + 1) * NH] = res.results[core]["out"]
    return full.reshape(B, C, hh, ww)


# revision 22
# speedup vs baseline: 1.1029x; 1.0553x over previous
# Trainium2 Bass kernel for NonLocalBlock (GroupNorm + 1x1-conv self-attention + residual).
#
# Full input x: [4, 256, 64, 64] f32. Output: x + proj(attn(gn(x))), same shape.
#
# Sharding: 8 cores = 4 batches x 2 query-halves. Attention is independent per
# batch; within a batch, softmax rows (queries) split cleanly across 2 cores.
# Each core redundantly computes GroupNorm + K + V^T for its batch (cheap), and
# computes scores/softmax/PV/proj only for its 2048 queries. No collectives.
#
# Per-core program (c = 256 channels as 2 partition-tiles, n = 4096 keys):
#   - GroupNorm stats: bn_stats/bn_aggr per channel, group-combine and
#     broadcast-back via tiny PE matmuls with 0/1 group matrices.
#   - h = x*A + B (bf16), plus the query half from a separate input slice so
#     all access patterns stay static across the SPMD program.
#   - k, q, vT in fp8-e4m3 with the contraction dim stored channel-interleaved
#     ([128, 2, *]), so the attention matmuls run in DoubleRow perf mode
#     (2 fp8 MACs/cell/cycle, K=256 per instruction). The interleave is
#     produced for free: host permutes weight columns; PSUM->SBUF copies land
#     each output-channel half in its pair plane.
#   - scores transposed: sT[j,i] = k^T q; exp on ACT fused with the
#     PSUM->SBUF copy (1/sqrt(c) folded into the activation scale); eT[j,i]
#     is then directly the PV moving operand - no transposes anywhere.
#   - row sums of exp via ones-vector DR matmuls; softmax normalization is a
#     column scaling that commutes through PV and proj, applied in the output
#     stage (reciprocal_approx_fast on a broadcast of the sums).
#   - bv never applied on-chip: softmax rows sum to 1, so wproj@bv folds into
#     bproj on the host. out = x_half + rinv * (wproj @ A_unnorm) + bproj_eff.
#
# Stationary-operand reuse: each k/vT slice serves all 4 query blocks
# back-to-back, so LDWEIGHTS is paid once per 4 matmuls.

import os
import sys

for _p in ("/opt/trn_rl_repo", "/root/.axon_site/_ro/trn_rl_repo"):
    if os.path.isdir(_p) and _p not in sys.path:
        sys.path.insert(0, _p)

import numpy as np
import ml_dtypes

import concourse.bass as bass
import concourse.tile as tile
from concourse import bacc, mybir
from concourse.alu_op_type import AluOpType
from concourse.bass_utils import run_bass_kernel_spmd

from concourse import dve_ops as _dvo
from concourse.dve_spec import Spec as _DveSpec, Src0 as _Src0, C0 as _C0, \
    C1 as _C1, C2 as _C2, sq as _sq, lower as _dve_lower
from concourse.dve_uop import DveOpSpec as _DveOpSpec
from concourse.dve_table_gen import dve_ver_for as _dve_ver_for


def _register_exp_q4():
    # out = (c2 + z*(c1 + z*c0))^4 ~= exp(z*s) when (c0,c1,c2) are the
    # quadratic Taylor of exp(z*s/4): lets the Vector engine carry half the
    # softmax exp load (single fused uop; ACT is otherwise the bottleneck).
    if "EXP_Q4_ANT" in _dvo._SUB_OPCODE_FOR_NAME:
        return next(op for op in _dvo.OPS if op.name == "EXP_Q4_ANT")
    ver = _dve_ver_for("TRN2")
    spec = _DveSpec(
        body=_sq(_sq(_C2 + _Src0 * (_C1 + _Src0 * _C0))),
        reference=lambda in0, in1, c0, c1, c2: (c2 + in0 * (c1 + in0 * c0)) ** 4)
    opcode = max(_dvo._SUB_OPCODE_FOR_NAME.values()) + 1
    sha = _DveOpSpec(name="EXP_Q4_ANT", opcode=opcode,
                     uops=_dve_lower(spec, ver=ver), rd1_en=False).sha(ver)
    op = _dvo.DveOp("EXP_Q4_ANT", spec, subdim=False, uops_sha={ver: sha})
    _dvo.OPS.append(op)
    _dvo.CUSTOM_DVE_SPECS["EXP_Q4_ANT"] = spec
    _dvo._SUB_OPCODE_FOR_NAME["EXP_Q4_ANT"] = opcode
    return op


EXP_Q4 = _register_exp_q4()

F32 = mybir.dt.float32
BF16 = mybir.dt.bfloat16
F8 = mybir.dt.float8e4
AF = mybir.ActivationFunctionType
DR = mybir.MatmulPerfMode.DoubleRow

B = 4
C = 256
N = 4096           # 64*64 spatial positions
NH = N // 2        # queries per core
GROUPS = 32
GSIZE = C // GROUPS  # 8 channels per group
EPS = 1e-6
P = 128
CT = C // P        # 2 channel tiles
JT = N // P        # 32 key tiles (16 DoubleRow pairs)
JP = JT // 2
NB = NH // 512     # 4 query blocks of 512
NCORES = 8
SCALE = float(1.0 / np.sqrt(C))

_cache = {}


def _col(ap_1d, ct):
    # View a [256] DRAM tensor as [256, 1] and take channel-tile ct's [128, 1].
    return ap_1d.ap().rearrange("(a b) -> a b", b=1)[ct * P:(ct + 1) * P, :]


def _build_program():
    nc = bacc.Bacc("TRN2", target_bir_lowering=False, debug=False)

    x_full = nc.dram_tensor("x_full", [C, N], F32, kind="ExternalInput")
    xh = nc.dram_tensor("xh", [C, NH], F32, kind="ExternalInput")
    gnsc = nc.dram_tensor("gnsc", [C], F32, kind="ExternalInput")
    gnbs = nc.dram_tensor("gnbs", [C], F32, kind="ExternalInput")
    g8 = nc.dram_tensor("g8", [P, P // GSIZE], F32, kind="ExternalInput")
    gt01 = nc.dram_tensor("gt01", [P // GSIZE, P], F32, kind="ExternalInput")
    wqT = nc.dram_tensor("wqT", [C, C], BF16, kind="ExternalInput")
    bq = nc.dram_tensor("bq", [C], F32, kind="ExternalInput")
    wkT = nc.dram_tensor("wkT", [C, C], BF16, kind="ExternalInput")
    bk = nc.dram_tensor("bk", [C], F32, kind="ExternalInput")
    wvT = nc.dram_tensor("wvT", [C, C], BF16, kind="ExternalInput")
    wpT = nc.dram_tensor("wpT", [C, C], BF16, kind="ExternalInput")
    bpe = nc.dram_tensor("bpe", [C], F32, kind="ExternalInput")
    out = nc.dram_tensor("out", [C, NH], F32, kind="ExternalOutput")
    rinv_scr = nc.dram_tensor("rinv_scr", [NH], F32)

    with tile.TileContext(nc) as tc:
        _body(tc, x_full, xh, gnsc, gnbs, g8, gt01,
              wqT, bq, wkT, bk, wvT, wpT, bpe, out, rinv_scr)
    nc.compile()
    return nc


def _body(tc, x_full, xh, gnsc, gnbs, g8, gt01,
          wqT, bq, wkT, bk, wvT, wpT, bpe, out, rinv_scr):
    nc = tc.nc
    NG = P // GSIZE  # 16 groups per channel tile

    from contextlib import ExitStack
    with ExitStack() as ctx:
        consts = ctx.enter_context(tc.tile_pool(name="consts", bufs=1))
        px = ctx.enter_context(tc.tile_pool(name="px", bufs=1))
        ph = ctx.enter_context(tc.tile_pool(name="ph", bufs=1))
        pkv = ctx.enter_context(tc.tile_pool(name="pkv", bufs=1))
        pst = ctx.enter_context(tc.tile_pool(name="pst", bufs=4))
        pout = ctx.enter_context(tc.tile_pool(name="pout", bufs=3))
        # PSUM: two 2-bank score/misc slots + two 2-bank PV accumulators = 8
        ps_big = ctx.enter_context(tc.tile_pool(name="ps_big", bufs=2, space="PSUM"))
        ps_sum = ps_big

        # ---- x load first: one 1MB DMA per (ct, half), two queues ----
        x_sb = []
        for ct in range(CT):
            xt = px.tile([P, N], F32, tag=f"x{ct}", name=f"x{ct}")
            for c2 in range(2):
                [nc.sync, nc.scalar, nc.gpsimd, nc.sync][ct * 2 + c2].dma_start(
                    out=xt[:, c2 * 2048:(c2 + 1) * 2048],
                    in_=x_full.ap()[ct * P:(ct + 1) * P, c2 * 2048:(c2 + 1) * 2048])
            x_sb.append(xt)

        # ---- constants ----
        # DR weights need 16B-aligned pair-plane step; pad the ones vector
        ones8_t = consts.tile([P, 2, 16], F8, tag="ones")
        nc.vector.memset(ones8_t, 1.0)
        ones8 = ones8_t[:, :, 0:1]
        g8_sb = consts.tile([P, NG], F32, tag="g8")
        nc.sync.dma_start(out=g8_sb, in_=g8.ap())
        gt01_sb = consts.tile([NG, P], F32, tag="gt01")
        nc.scalar.dma_start(out=gt01_sb, in_=gt01.ap())

        w_sb = {}
        for name, h in (("wqT", wqT), ("wkT", wkT), ("wvT", wvT), ("wpT", wpT)):
            for ec in range(CT):
                t = consts.tile([P, C], BF16, tag=f"{name}{ec}")
                [nc.sync, nc.scalar][ec].dma_start(
                    out=t, in_=h.ap()[ec * P:(ec + 1) * P, :])
                w_sb[(name, ec)] = t

        col_sb = {}
        for name, h in (("gnsc", gnsc), ("gnbs", gnbs), ("bq", bq),
                        ("bk", bk), ("bpe", bpe)):
            for ct in range(CT):
                t = consts.tile([P, 1], F32, tag=f"{name}{ct}")
                [nc.sync, nc.scalar][ct].dma_start(out=t, in_=_col(h, ct))
                col_sb[(name, ct)] = t

        # ---- GroupNorm stats ----
        ab_cols = []
        for ct in range(CT):
            xt = x_sb[ct]
            stats = pst.tile([P, 8, nc.vector.BN_STATS_DIM], F32, tag="bnst")
            for s in range(8):
                nc.vector.bn_stats(out=stats[:, s, :], in_=xt[:, s * 512:(s + 1) * 512])
            mv = pst.tile([P, nc.vector.BN_AGGR_DIM], F32, tag="bnagg")
            nc.vector.bn_aggr(out=mv, in_=stats)

            # per-channel (mean, E[x^2]) -> per-group via G/8 matmul
            st2 = pst.tile([P, 2], F32, tag="st2")
            nc.vector.tensor_copy(out=st2[:, 0:1], in_=mv[:, 0:1])
            m2 = pst.tile([P, 1], F32, tag="m2")
            nc.vector.tensor_mul(m2, mv[:, 0:1], mv[:, 0:1])
            nc.vector.tensor_add(st2[:, 1:2], m2, mv[:, 1:2])

            gps = ps_big.tile([NG, 2], F32, tag="big")
            nc.tensor.matmul(gps, lhsT=g8_sb, rhs=st2, start=True, stop=True)
            gs = pst.tile([NG, 2], F32, tag="gs")
            nc.vector.tensor_copy(out=gs, in_=gps)

            # var_g = E[x^2]_g - mean_g^2 ; rstd = 1/sqrt(var+eps)
            vg = pst.tile([NG, 1], F32, tag="vg")
            nc.vector.tensor_mul(vg, gs[:, 0:1], gs[:, 0:1])
            nc.vector.tensor_tensor(out=vg, in0=gs[:, 1:2], in1=vg,
                                    op=AluOpType.subtract)
            eps_t = pst.tile([NG, 1], F32, tag="eps")
            nc.vector.memset(eps_t, EPS)
            std = pst.tile([NG, 1], F32, tag="std")
            nc.scalar.activation(out=std, in_=vg, func=AF.Sqrt, bias=eps_t, scale=1.0)
            rstd = pst.tile([NG, 1], F32, tag="rstd")
            nc.vector.reciprocal(out=rstd, in_=std)

            gs2 = pst.tile([NG, 2], F32, tag="gs2")
            nc.vector.tensor_copy(out=gs2[:, 0:1], in_=gs[:, 0:1])
            nc.vector.tensor_copy(out=gs2[:, 1:2], in_=rstd)

            # broadcast (mean_g, rstd_g) back to channels
            bps = ps_big.tile([P, 2], F32, tag="big")
            nc.tensor.matmul(bps, lhsT=gt01_sb, rhs=gs2, start=True, stop=True)
            mr = pst.tile([P, 2], F32, tag="mr")
            nc.vector.tensor_copy(out=mr, in_=bps)

            a_col = pst.tile([P, 1], F32, tag=f"acol{ct}")
            nc.vector.tensor_mul(a_col, mr[:, 1:2], col_sb[("gnsc", ct)])
            b_col = pst.tile([P, 1], F32, tag=f"bcol{ct}")
            nc.vector.tensor_mul(b_col, mr[:, 0:1], a_col)
            nc.vector.tensor_tensor(out=b_col, in0=col_sb[("gnbs", ct)],
                                    in1=b_col, op=AluOpType.subtract)
            ab_cols.append((a_col, b_col))

        # ---- h = x*A+B (chunked so k/vT matmuls start early); k, vT ----
        # k_sb/q_sb/vT_dr are fp8 with channels pair-interleaved for DoubleRow:
        # value (p, q, .) = channel 2p+q (host permuted the weight columns).
        h_sb = [ph.tile([P, N], BF16, tag=f"h{ct}", name=f"h{ct}") for ct in range(CT)]
        k_sb = pkv.tile([P, 2, N], F8, tag="k")
        vT_dr = pkv.tile([P, 2, JP, C], F8, tag="vT")
        for c4 in range(4):
            j0 = c4 * 1024
            for ct in range(CT):
                a_col, b_col = ab_cols[ct]
                nc.gpsimd.tensor_scalar(
                    out=h_sb[ct][:, j0:j0 + 1024], in0=x_sb[ct][:, j0:j0 + 1024],
                    scalar1=a_col, scalar2=b_col,
                    op0=AluOpType.mult, op1=AluOpType.add)
            for dt in range(CT):
                ps = ps_big.tile([P, 1024], F32, tag=["big", "pva"][(c4 + dt) % 2], name=f"k{c4}_{dt}")
                for jj in range(2):
                    jc = 2 * c4 + jj
                    for ec in range(CT):
                        nc.tensor.matmul(
                            ps[:, jj * 512:(jj + 1) * 512],
                            lhsT=w_sb[("wkT", ec)][:, dt * P:(dt + 1) * P],
                            rhs=h_sb[ec][:, jc * 512:(jc + 1) * 512],
                            start=(ec == 0), stop=(ec == CT - 1))
                nc.scalar.activation(
                    out=k_sb[:, dt, j0:j0 + 1024], in_=ps,
                    func=AF.Identity, bias=col_sb[("bk", dt)], scale=1.0)
            for t in (2 * c4, 2 * c4 + 1):
                # four jt per psum tile, quarters ordered (q, jtp) so one copy
                # lands them all in vT_dr[:, :, 2t:2t+2, :]
                ps = ps_big.tile([P, 4, C], F32, tag=["big", "pva"][t % 2],
                                 name=f"v{t}")
                for jj in range(4):
                    jt = 4 * t + jj
                    quarter = (jt % 2) * 2 + (jt // 2) % 2
                    for ec in range(CT):
                        nc.tensor.matmul(
                            ps[:, quarter, :],
                            lhsT=h_sb[ec][:, jt * P:(jt + 1) * P],
                            rhs=w_sb[("wvT", ec)],
                            start=(ec == 0), stop=(ec == CT - 1))
                nc.vector.tensor_copy(out=vT_dr[:, :, 2 * t:2 * t + 2, :],
                                      in_=ps)

        # ---- query-half h, q ----
        xh_sb, hh_sb = [], []
        for ct in range(CT):
            xht = px.tile([P, NH], F32, tag=f"x{ct}", name=f"xh{ct}")
            [nc.scalar, nc.sync][ct].dma_start(
                out=xht, in_=xh.ap()[ct * P:(ct + 1) * P, :])
            xh_sb.append(xht)
            a_col, b_col = ab_cols[ct]
            hht = ph.tile([P, NH], BF16, tag=f"hh{ct}", name=f"hh{ct}")
            nc.gpsimd.tensor_scalar(out=hht, in0=xht, scalar1=a_col, scalar2=b_col,
                                    op0=AluOpType.mult, op1=AluOpType.add)
            hh_sb.append(hht)

        q_sb = pkv.tile([P, 2, NH], F8, tag="q")
        for dt in range(CT):
            for icp in range(2):
                ps = ps_big.tile([P, 1024], F32, tag=["big", "pva"][(dt + icp) % 2], name=f"q{dt}_{icp}")
                for ii in range(2):
                    ic = 2 * icp + ii
                    for ec in range(CT):
                        nc.tensor.matmul(
                            ps[:, ii * 512:(ii + 1) * 512],
                            lhsT=w_sb[("wqT", ec)][:, dt * P:(dt + 1) * P],
                            rhs=hh_sb[ec][:, ic * 512:(ic + 1) * 512],
                            start=(ec == 0), stop=(ec == CT - 1))
                nc.scalar.activation(
                    out=q_sb[:, dt, icp * 1024:(icp + 1) * 1024], in_=ps,
                    func=AF.Identity, bias=col_sb[("bq", dt)], scale=1.0)

        # ---- attention: one pass over the 32 key tiles for all 2048
        # queries. Per key tile: 4 DR score matmuls; exp of the first query
        # half on ACT, of the second half on the Vector engine (fused q^4
        # polynomial - splitting exp across engines is what keeps PE fed).
        # PV for channel-tile 0 rides along; sums/PV-ct1/proj follow.
        # eT[p, jtp, q, i] = exp(s[j=(2*jtp+q)*128+p, i]/16)  (fp8)
        eT = pkv.tile([P, JP, 2, NH], F8, tag="eT")
        A_sb = [pkv.tile([P, NH], BF16, tag=f"A{ct}", name=f"A{ct}")
                for ct in range(CT)]
        rinvb = pkv.tile([P, NH], F32, tag="rinvb")
        EC1 = SCALE / 4.0
        EC0 = SCALE * SCALE / 32.0

        # scores: four rotating psum slots (both tag groups) so the two exp
        # engines pipeline freely; per jt, half0 exps on ACT, half1 on DVE
        for jt in range(JT):
            kw = k_sb[:, :, jt * P:(jt + 1) * P]
            for half in range(2):
                ps = ps_big.tile([P, 1024], F32,
                                 tag=["big", "pva"][jt % 2],
                                 name=f"sc{jt}_{half}")
                for ii in range(2):
                    ib = 2 * half + ii
                    nc.tensor.matmul(
                        ps[:, ii * 512:(ii + 1) * 512], lhsT=kw,
                        rhs=q_sb[:, :, ib * 512:(ib + 1) * 512],
                        start=True, stop=True, perf_mode=DR)
                dst = eT[:, jt // 2, jt % 2, half * 1024:(half + 1) * 1024]
                if half == 0:
                    nc.scalar.activation(out=dst, in_=ps, func=AF.Exp,
                                         scale=SCALE)
                else:
                    nc.vector._custom_dve(EXP_Q4, out=dst, in0=ps,
                                          s0=EC0, s1=EC1, imm2=1.0)

        # row sums: 4 accumulators spread over both tag groups
        pssums = [ps_big.tile([1, 512], F32, tag=["big", "pva"][ib % 2],
                              name=f"sm{ib}") for ib in range(NB)]
        for jtp in range(JP):
            for ib in range(NB):
                nc.tensor.matmul(pssums[ib], lhsT=ones8,
                                 rhs=eT[:, jtp, :, ib * 512:(ib + 1) * 512],
                                 start=(jtp == 0), stop=(jtp == JP - 1),
                                 perf_mode=DR)
        for ib in range(NB):
            i0 = ib * 512
            srow = pst.tile([1, 512], F32, tag="srow")
            nc.vector.tensor_copy(out=srow, in_=pssums[ib])
            nc.sync.dma_start(
                out=rinv_scr.ap().rearrange("(a b) -> a b", a=1)[:, i0:i0 + 512],
                in_=srow)
            rsc = rinv_scr.ap()[i0:i0 + 512]
            sb = pout.tile([P, 512], F32, tag="sb")
            nc.gpsimd.dma_start(
                out=sb,
                in_=bass.AP(tensor=rsc.tensor, offset=rsc.offset,
                            ap=[[0, P]] + [list(d) for d in rsc.ap]))
            nc.vector.reciprocal_approx_fast(out=rinvb[:, i0:i0 + 512],
                                             in_=sb)

        # PV: both channel tiles accumulate concurrently (8 banks), each
        # vT slice stationary across 4 matmuls
        psas = {(ct, h): ps_big.tile([P, 1024], F32,
                                     tag=["big", "pva"][ct],
                                     name=f"pv{ct}_{h}")
                for ct in range(CT) for h in range(2)}
        for jtp in range(JP):
            for ct in range(CT):
                vw = vT_dr[:, :, jtp, ct * P:(ct + 1) * P]
                for half in range(2):
                    for ii in range(2):
                        ib = 2 * half + ii
                        nc.tensor.matmul(
                            psas[(ct, half)][:, ii * 512:(ii + 1) * 512],
                            lhsT=vw,
                            rhs=eT[:, jtp, :, ib * 512:(ib + 1) * 512],
                            start=(jtp == 0), stop=(jtp == JP - 1),
                            perf_mode=DR)
        for ct in range(CT):
            for half in range(2):
                nc.scalar.activation(
                    out=A_sb[ct][:, half * 1024:(half + 1) * 1024],
                    in_=psas[(ct, half)], func=AF.Copy)

        # ---- output projection + normalization + bias + residual ----
        for dt in range(CT):
            for icp in range(2):
                i0 = icp * 1024
                ps = ps_big.tile([P, 1024], F32, tag=["big", "pva"][(dt + icp) % 2],
                                 name=f"pj{dt}_{icp}")
                for ii in range(2):
                    ic = 2 * icp + ii
                    for cc in range(CT):
                        nc.tensor.matmul(
                            ps[:, ii * 512:(ii + 1) * 512],
                            lhsT=w_sb[("wpT", cc)][:, dt * P:(dt + 1) * P],
                            rhs=A_sb[cc][:, ic * 512:(ic + 1) * 512],
                            start=(cc == 0), stop=(cc == CT - 1))
                ot = pout.tile([P, 1024], F32, tag="ot")
                nc.vector.tensor_mul(ot, ps, rinvb[:, i0:i0 + 1024])
                nc.vector.scalar_tensor_tensor(
                    out=ot, in0=ot, scalar=col_sb[("bpe", dt)],
                    in1=xh_sb[dt][:, i0:i0 + 1024],
                    op0=AluOpType.add, op1=AluOpType.add)
                nc.sync.dma_start(
                    out=out.ap()[dt * P:(dt + 1) * P, i0:i0 + 1024],
                    in_=ot)


def _get_program():
    if "nc" not in _cache:
        _cache["nc"] = _build_program()
    return _cache["nc"]


def kernel(x, gn_scale, gn_bias, wq, bq, wk, bk, wv, bv, wproj, bproj):
    x = np.asarray(x, dtype=np.float32)
    b, c, hh, ww = x.shape
    assert (b, c, hh * ww) == (B, C, N)
    xf = np.ascontiguousarray(x.reshape(B, C, N))

    bf = ml_dtypes.bfloat16
    # Channel-pair interleave permutation for DoubleRow: even channels then odd.
    perm = np.concatenate([np.arange(0, C, 2), np.arange(1, C, 2)])
    wqT_s = np.ascontiguousarray(np.asarray(wq, np.float32).T[:, perm]).astype(bf)
    bq_s = np.ascontiguousarray(np.asarray(bq, np.float32)[perm])
    wkT = np.ascontiguousarray(np.asarray(wk, np.float32).T[:, perm]).astype(bf)
    bk_s = np.ascontiguousarray(np.asarray(bk, np.float32)[perm])
    wvT = np.ascontiguousarray(np.asarray(wv, np.float32).T[:, perm]).astype(bf)
    wpT = np.ascontiguousarray(np.asarray(wproj, np.float32).T[perm, :]).astype(bf)
    # softmax rows sum to 1 => v-bias contributes wproj@bv, constant per channel
    bpe = (np.asarray(bproj, np.float64)
           + np.asarray(wproj, np.float64) @ np.asarray(bv, np.float64)
           ).astype(np.float32)

    g8 = np.zeros((P, P // GSIZE), np.float32)
    gt01 = np.zeros((P // GSIZE, P), np.float32)
    for ch in range(P):
        g8[ch, ch // GSIZE] = 1.0 / GSIZE   # yields per-group means directly
        gt01[ch // GSIZE, ch] = 1.0

    common = dict(gnsc=np.asarray(gn_scale, np.float32),
                  gnbs=np.asarray(gn_bias, np.float32),
                  g8=g8, gt01=gt01,
                  wqT=wqT_s, bq=bq_s, wkT=wkT, bk=bk_s,
                  wvT=wvT, wpT=wpT, bpe=bpe)

    in_maps = []
    for core in range(NCORES):
        bi, half = core // 2, core % 2
        in_maps.append(dict(
            x_full=np.ascontiguousarray(xf[bi]),
            xh=np.ascontiguousarray(xf[bi][:, half * NH:(half + 1) * NH]),
            **common))

    nc = _get_program()
    trace = bool(os.environ.get("BASS_KERNEL_TRACE"))
    res = run_bass_kernel_spmd(nc, in_maps, core_ids=list(range(NCORES)),
                               trace=trace)
    _cache["last_results"] = res

    full = np.empty((B, C, N), np.float32)
    for core in range(NCORES):
        bi, half = core // 2, core % 2
        full[bi][:, half * NH:(half + 1) * NH] = res.results[core]["out"]
    return full.reshape(B, C, hh, ww)


# revision 23
# speedup vs baseline: 1.1097x; 1.0062x over previous
# Trainium2 Bass kernel for NonLocalBlock (GroupNorm + 1x1-conv self-attention + residual).
#
# Full input x: [4, 256, 64, 64] f32. Output: x + proj(attn(gn(x))), same shape.
#
# Sharding: 8 cores = 4 batches x 2 query-halves. Attention is independent per
# batch; within a batch, softmax rows (queries) split cleanly across 2 cores.
# Each core redundantly computes GroupNorm + K + V^T for its batch (cheap), and
# computes scores/softmax/PV/proj only for its 2048 queries. No collectives.
#
# Per-core program (c = 256 channels as 2 partition-tiles, n = 4096 keys):
#   - GroupNorm stats: bn_stats/bn_aggr per channel, group-combine and
#     broadcast-back via tiny PE matmuls with 0/1 group matrices.
#   - h = x*A + B (bf16), plus the query half from a separate input slice so
#     all access patterns stay static across the SPMD program.
#   - k, q, vT in fp8-e4m3 with the contraction dim stored channel-interleaved
#     ([128, 2, *]), so the attention matmuls run in DoubleRow perf mode
#     (2 fp8 MACs/cell/cycle, K=256 per instruction). The interleave is
#     produced for free: host permutes weight columns; PSUM->SBUF copies land
#     each output-channel half in its pair plane.
#   - scores transposed: sT[j,i] = k^T q; exp on ACT fused with the
#     PSUM->SBUF copy (1/sqrt(c) folded into the activation scale); eT[j,i]
#     is then directly the PV moving operand - no transposes anywhere.
#   - row sums of exp via ones-vector DR matmuls; softmax normalization is a
#     column scaling that commutes through PV and proj, applied in the output
#     stage (reciprocal_approx_fast on a broadcast of the sums).
#   - bv never applied on-chip: softmax rows sum to 1, so wproj@bv folds into
#     bproj on the host. out = x_half + rinv * (wproj @ A_unnorm) + bproj_eff.
#
# Stationary-operand reuse: each k/vT slice serves all 4 query blocks
# back-to-back, so LDWEIGHTS is paid once per 4 matmuls.

import os
import sys

for _p in ("/opt/trn_rl_repo", "/root/.axon_site/_ro/trn_rl_repo"):
    if os.path.isdir(_p) and _p not in sys.path:
        sys.path.insert(0, _p)

import numpy as np
import ml_dtypes

import concourse.bass as bass
import concourse.tile as tile
from concourse import bacc, mybir
from concourse.alu_op_type import AluOpType
from concourse.bass_utils import run_bass_kernel_spmd

from concourse import dve_ops as _dvo
from concourse.dve_spec import Spec as _DveSpec, Src0 as _Src0, C0 as _C0, \
    C1 as _C1, C2 as _C2, sq as _sq, lower as _dve_lower
from concourse.dve_uop import DveOpSpec as _DveOpSpec
from concourse.dve_table_gen import dve_ver_for as _dve_ver_for


def _register_exp_q4():
    # out = (c2 + z*(c1 + z*c0))^4 ~= exp(z*s) when (c0,c1,c2) are the
    # quadratic Taylor of exp(z*s/4): lets the Vector engine carry half the
    # softmax exp load (single fused uop; ACT is otherwise the bottleneck).
    if "EXP_Q4_ANT" in _dvo._SUB_OPCODE_FOR_NAME:
        return next(op for op in _dvo.OPS if op.name == "EXP_Q4_ANT")
    ver = _dve_ver_for("TRN2")
    spec = _DveSpec(
        body=_sq(_sq(_C2 + _Src0 * (_C1 + _Src0 * _C0))),
        reference=lambda in0, in1, c0, c1, c2: (c2 + in0 * (c1 + in0 * c0)) ** 4)
    opcode = max(_dvo._SUB_OPCODE_FOR_NAME.values()) + 1
    sha = _DveOpSpec(name="EXP_Q4_ANT", opcode=opcode,
                     uops=_dve_lower(spec, ver=ver), rd1_en=False).sha(ver)
    op = _dvo.DveOp("EXP_Q4_ANT", spec, subdim=False, uops_sha={ver: sha})
    _dvo.OPS.append(op)
    _dvo.CUSTOM_DVE_SPECS["EXP_Q4_ANT"] = spec
    _dvo._SUB_OPCODE_FOR_NAME["EXP_Q4_ANT"] = opcode
    return op


EXP_Q4 = _register_exp_q4()

F32 = mybir.dt.float32
BF16 = mybir.dt.bfloat16
F8 = mybir.dt.float8e4
AF = mybir.ActivationFunctionType
DR = mybir.MatmulPerfMode.DoubleRow

B = 4
C = 256
N = 4096           # 64*64 spatial positions
NH = N // 2        # queries per core
GROUPS = 32
GSIZE = C // GROUPS  # 8 channels per group
EPS = 1e-6
P = 128
CT = C // P        # 2 channel tiles
JT = N // P        # 32 key tiles (16 DoubleRow pairs)
JP = JT // 2
NB = NH // 512     # 4 query blocks of 512
NCORES = 8
SCALE = float(1.0 / np.sqrt(C))

_cache = {}


def _col(ap_1d, ct):
    # View a [256] DRAM tensor as [256, 1] and take channel-tile ct's [128, 1].
    return ap_1d.ap().rearrange("(a b) -> a b", b=1)[ct * P:(ct + 1) * P, :]


def _build_program():
    nc = bacc.Bacc("TRN2", target_bir_lowering=False, debug=False)

    x_full = nc.dram_tensor("x_full", [C, N], F32, kind="ExternalInput")
    xh = nc.dram_tensor("xh", [C, NH], F32, kind="ExternalInput")
    gnsc = nc.dram_tensor("gnsc", [C], F32, kind="ExternalInput")
    gnbs = nc.dram_tensor("gnbs", [C], F32, kind="ExternalInput")
    g8 = nc.dram_tensor("g8", [P, P // GSIZE], F32, kind="ExternalInput")
    gt01 = nc.dram_tensor("gt01", [P // GSIZE, P], F32, kind="ExternalInput")
    wqT = nc.dram_tensor("wqT", [C, C], BF16, kind="ExternalInput")
    bq = nc.dram_tensor("bq", [C], F32, kind="ExternalInput")
    wkT = nc.dram_tensor("wkT", [C, C], BF16, kind="ExternalInput")
    bk = nc.dram_tensor("bk", [C], F32, kind="ExternalInput")
    wvT = nc.dram_tensor("wvT", [C, C], BF16, kind="ExternalInput")
    wpT = nc.dram_tensor("wpT", [C, C], BF16, kind="ExternalInput")
    bpe = nc.dram_tensor("bpe", [C], F32, kind="ExternalInput")
    out = nc.dram_tensor("out", [C, NH], F32, kind="ExternalOutput")
    rinv_scr = nc.dram_tensor("rinv_scr", [NH], F32)

    with tile.TileContext(nc) as tc:
        _body(tc, x_full, xh, gnsc, gnbs, g8, gt01,
              wqT, bq, wkT, bk, wvT, wpT, bpe, out, rinv_scr)
    nc.compile()
    return nc


def _body(tc, x_full, xh, gnsc, gnbs, g8, gt01,
          wqT, bq, wkT, bk, wvT, wpT, bpe, out, rinv_scr):
    nc = tc.nc
    NG = P // GSIZE  # 16 groups per channel tile

    from contextlib import ExitStack
    with ExitStack() as ctx:
        consts = ctx.enter_context(tc.tile_pool(name="consts", bufs=1))
        px = ctx.enter_context(tc.tile_pool(name="px", bufs=1))
        ph = ctx.enter_context(tc.tile_pool(name="ph", bufs=1))
        pkv = ctx.enter_context(tc.tile_pool(name="pkv", bufs=1))
        pst = ctx.enter_context(tc.tile_pool(name="pst", bufs=4))
        pout = ctx.enter_context(tc.tile_pool(name="pout", bufs=3))
        # PSUM: two 2-bank score/misc slots + two 2-bank PV accumulators = 8
        ps_big = ctx.enter_context(tc.tile_pool(name="ps_big", bufs=2, space="PSUM"))
        ps_sum = ps_big

        # ---- x load first: 16 x 256KB pieces round-robin over all three
        # DMA queues, interleaved across channel tiles so bn_stats (one per
        # 512-column chunk) starts as each piece lands ----
        x_sb = [px.tile([P, N], F32, tag=f"x{ct}", name=f"x{ct}")
                for ct in range(CT)]
        qs = [nc.sync, nc.scalar, nc.gpsimd]
        pi = 0
        for s in range(8):
            for ct in range(CT):
                qs[pi % 3].dma_start(
                    out=x_sb[ct][:, s * 512:(s + 1) * 512],
                    in_=x_full.ap()[ct * P:(ct + 1) * P, s * 512:(s + 1) * 512])
                pi += 1

        # ---- constants ----
        # DR weights need 16B-aligned pair-plane step; pad the ones vector
        ones8_t = consts.tile([P, 2, 16], F8, tag="ones")
        nc.vector.memset(ones8_t, 1.0)
        ones8 = ones8_t[:, :, 0:1]
        g8_sb = consts.tile([P, NG], F32, tag="g8")
        nc.sync.dma_start(out=g8_sb, in_=g8.ap())
        gt01_sb = consts.tile([NG, P], F32, tag="gt01")
        nc.scalar.dma_start(out=gt01_sb, in_=gt01.ap())

        w_sb = {}
        for name, h in (("wqT", wqT), ("wkT", wkT), ("wvT", wvT), ("wpT", wpT)):
            for ec in range(CT):
                t = consts.tile([P, C], BF16, tag=f"{name}{ec}")
                [nc.sync, nc.scalar][ec].dma_start(
                    out=t, in_=h.ap()[ec * P:(ec + 1) * P, :])
                w_sb[(name, ec)] = t

        col_sb = {}
        for name, h in (("gnsc", gnsc), ("gnbs", gnbs), ("bq", bq),
                        ("bk", bk), ("bpe", bpe)):
            for ct in range(CT):
                t = consts.tile([P, 1], F32, tag=f"{name}{ct}")
                [nc.sync, nc.scalar][ct].dma_start(out=t, in_=_col(h, ct))
                col_sb[(name, ct)] = t

        # ---- GroupNorm stats ----
        ab_cols = []
        for ct in range(CT):
            xt = x_sb[ct]
            stats = pst.tile([P, 8, nc.vector.BN_STATS_DIM], F32, tag="bnst")
            for s in range(8):
                nc.vector.bn_stats(out=stats[:, s, :], in_=xt[:, s * 512:(s + 1) * 512])
            mv = pst.tile([P, nc.vector.BN_AGGR_DIM], F32, tag="bnagg")
            nc.vector.bn_aggr(out=mv, in_=stats)

            # per-channel (mean, E[x^2]) -> per-group via G/8 matmul
            st2 = pst.tile([P, 2], F32, tag="st2")
            nc.vector.tensor_copy(out=st2[:, 0:1], in_=mv[:, 0:1])
            m2 = pst.tile([P, 1], F32, tag="m2")
            nc.vector.tensor_mul(m2, mv[:, 0:1], mv[:, 0:1])
            nc.vector.tensor_add(st2[:, 1:2], m2, mv[:, 1:2])

            gps = ps_big.tile([NG, 2], F32, tag="big")
            nc.tensor.matmul(gps, lhsT=g8_sb, rhs=st2, start=True, stop=True)
            gs = pst.tile([NG, 2], F32, tag="gs")
            nc.vector.tensor_copy(out=gs, in_=gps)

            # var_g = E[x^2]_g - mean_g^2 ; rstd = 1/sqrt(var+eps)
            vg = pst.tile([NG, 1], F32, tag="vg")
            nc.vector.tensor_mul(vg, gs[:, 0:1], gs[:, 0:1])
            nc.vector.tensor_tensor(out=vg, in0=gs[:, 1:2], in1=vg,
                                    op=AluOpType.subtract)
            eps_t = pst.tile([NG, 1], F32, tag="eps")
            nc.vector.memset(eps_t, EPS)
            std = pst.tile([NG, 1], F32, tag="std")
            nc.scalar.activation(out=std, in_=vg, func=AF.Sqrt, bias=eps_t, scale=1.0)
            rstd = pst.tile([NG, 1], F32, tag="rstd")
            nc.vector.reciprocal(out=rstd, in_=std)

            gs2 = pst.tile([NG, 2], F32, tag="gs2")
            nc.vector.tensor_copy(out=gs2[:, 0:1], in_=gs[:, 0:1])
            nc.vector.tensor_copy(out=gs2[:, 1:2], in_=rstd)

            # broadcast (mean_g, rstd_g) back to channels
            bps = ps_big.tile([P, 2], F32, tag="big")
            nc.tensor.matmul(bps, lhsT=gt01_sb, rhs=gs2, start=True, stop=True)
            mr = pst.tile([P, 2], F32, tag="mr")
            nc.vector.tensor_copy(out=mr, in_=bps)

            a_col = pst.tile([P, 1], F32, tag=f"acol{ct}")
            nc.vector.tensor_mul(a_col, mr[:, 1:2], col_sb[("gnsc", ct)])
            b_col = pst.tile([P, 1], F32, tag=f"bcol{ct}")
            nc.vector.tensor_mul(b_col, mr[:, 0:1], a_col)
            nc.vector.tensor_tensor(out=b_col, in0=col_sb[("gnbs", ct)],
                                    in1=b_col, op=AluOpType.subtract)
            ab_cols.append((a_col, b_col))

        # ---- h = x*A+B (chunked so k/vT matmuls start early); k, vT ----
        # k_sb/q_sb/vT_dr are fp8 with channels pair-interleaved for DoubleRow:
        # value (p, q, .) = channel 2p+q (host permuted the weight columns).
        h_sb = [ph.tile([P, N], BF16, tag=f"h{ct}", name=f"h{ct}") for ct in range(CT)]
        k_sb = pkv.tile([P, 2, N], F8, tag="k")
        vT_dr = pkv.tile([P, 2, JP, C], F8, tag="vT")
        for c4 in range(4):
            j0 = c4 * 1024
            for ct in range(CT):
                a_col, b_col = ab_cols[ct]
                nc.gpsimd.tensor_scalar(
                    out=h_sb[ct][:, j0:j0 + 1024], in0=x_sb[ct][:, j0:j0 + 1024],
                    scalar1=a_col, scalar2=b_col,
                    op0=AluOpType.mult, op1=AluOpType.add)
            for dt in range(CT):
                ps = ps_big.tile([P, 1024], F32, tag=["big", "pva"][(c4 + dt) % 2], name=f"k{c4}_{dt}")
                for jj in range(2):
                    jc = 2 * c4 + jj
                    for ec in range(CT):
                        nc.tensor.matmul(
                            ps[:, jj * 512:(jj + 1) * 512],
                            lhsT=w_sb[("wkT", ec)][:, dt * P:(dt + 1) * P],
                            rhs=h_sb[ec][:, jc * 512:(jc + 1) * 512],
                            start=(ec == 0), stop=(ec == CT - 1))
                nc.scalar.activation(
                    out=k_sb[:, dt, j0:j0 + 1024], in_=ps,
                    func=AF.Identity, bias=col_sb[("bk", dt)], scale=1.0)
            for t in (2 * c4, 2 * c4 + 1):
                # four jt per psum tile, quarters ordered (q, jtp) so one copy
                # lands them all in vT_dr[:, :, 2t:2t+2, :]
                ps = ps_big.tile([P, 4, C], F32, tag=["big", "pva"][t % 2],
                                 name=f"v{t}")
                for jj in range(4):
                    jt = 4 * t + jj
                    quarter = (jt % 2) * 2 + (jt // 2) % 2
                    for ec in range(CT):
                        nc.tensor.matmul(
                            ps[:, quarter, :],
                            lhsT=h_sb[ec][:, jt * P:(jt + 1) * P],
                            rhs=w_sb[("wvT", ec)],
                            start=(ec == 0), stop=(ec == CT - 1))
                nc.vector.tensor_copy(out=vT_dr[:, :, 2 * t:2 * t + 2, :],
                                      in_=ps)

        # ---- query-half h, q ----
        xh_sb, hh_sb = [], []
        for ct in range(CT):
            xht = px.tile([P, NH], F32, tag=f"x{ct}", name=f"xh{ct}")
            [nc.scalar, nc.sync][ct].dma_start(
                out=xht, in_=xh.ap()[ct * P:(ct + 1) * P, :])
            xh_sb.append(xht)
            a_col, b_col = ab_cols[ct]
            hht = ph.tile([P, NH], BF16, tag=f"hh{ct}", name=f"hh{ct}")
            nc.gpsimd.tensor_scalar(out=hht, in0=xht, scalar1=a_col, scalar2=b_col,
                                    op0=AluOpType.mult, op1=AluOpType.add)
            hh_sb.append(hht)

        q_sb = pkv.tile([P, 2, NH], F8, tag="q")
        for dt in range(CT):
            for icp in range(2):
                ps = ps_big.tile([P, 1024], F32, tag=["big", "pva"][(dt + icp) % 2], name=f"q{dt}_{icp}")
                for ii in range(2):
                    ic = 2 * icp + ii
                    for ec in range(CT):
                        nc.tensor.matmul(
                            ps[:, ii * 512:(ii + 1) * 512],
                            lhsT=w_sb[("wqT", ec)][:, dt * P:(dt + 1) * P],
                            rhs=hh_sb[ec][:, ic * 512:(ic + 1) * 512],
                            start=(ec == 0), stop=(ec == CT - 1))
                nc.scalar.activation(
                    out=q_sb[:, dt, icp * 1024:(icp + 1) * 1024], in_=ps,
                    func=AF.Identity, bias=col_sb[("bq", dt)], scale=1.0)

        # ---- attention: one pass over the 32 key tiles for all 2048
        # queries. Per key tile: 4 DR score matmuls; exp of the first query
        # half on ACT, of the second half on the Vector engine (fused q^4
        # polynomial - splitting exp across engines is what keeps PE fed).
        # PV for channel-tile 0 rides along; sums/PV-ct1/proj follow.
        # eT[p, jtp, q, i] = exp(s[j=(2*jtp+q)*128+p, i]/16)  (fp8)
        eT = pkv.tile([P, JP, 2, NH], F8, tag="eT")
        A_sb = [pkv.tile([P, NH], BF16, tag=f"A{ct}", name=f"A{ct}")
                for ct in range(CT)]
        rinvb = pkv.tile([P, NH], F32, tag="rinvb")
        EC1 = SCALE / 4.0
        EC0 = SCALE * SCALE / 32.0

        # scores: four rotating psum slots (both tag groups) so the two exp
        # engines pipeline freely; per jt, half0 exps on ACT, half1 on DVE
        for jt in range(JT):
            kw = k_sb[:, :, jt * P:(jt + 1) * P]
            for half in range(2):
                ps = ps_big.tile([P, 1024], F32,
                                 tag=["big", "pva"][jt % 2],
                                 name=f"sc{jt}_{half}")
                for ii in range(2):
                    ib = 2 * half + ii
                    nc.tensor.matmul(
                        ps[:, ii * 512:(ii + 1) * 512], lhsT=kw,
                        rhs=q_sb[:, :, ib * 512:(ib + 1) * 512],
                        start=True, stop=True, perf_mode=DR)
                dst = eT[:, jt // 2, jt % 2, half * 1024:(half + 1) * 1024]
                if half == 0:
                    nc.scalar.activation(out=dst, in_=ps, func=AF.Exp,
                                         scale=SCALE)
                else:
                    nc.vector._custom_dve(EXP_Q4, out=dst, in0=ps,
                                          s0=EC0, s1=EC1, imm2=1.0)

        # row sums: 4 accumulators spread over both tag groups
        pssums = [ps_big.tile([1, 512], F32, tag=["big", "pva"][ib % 2],
                              name=f"sm{ib}") for ib in range(NB)]
        for jtp in range(JP):
            for ib in range(NB):
                nc.tensor.matmul(pssums[ib], lhsT=ones8,
                                 rhs=eT[:, jtp, :, ib * 512:(ib + 1) * 512],
                                 start=(jtp == 0), stop=(jtp == JP - 1),
                                 perf_mode=DR)
        for ib in range(NB):
            i0 = ib * 512
            srow = pst.tile([1, 512], F32, tag="srow")
            nc.vector.tensor_copy(out=srow, in_=pssums[ib])
            nc.sync.dma_start(
                out=rinv_scr.ap().rearrange("(a b) -> a b", a=1)[:, i0:i0 + 512],
                in_=srow)
            rsc = rinv_scr.ap()[i0:i0 + 512]
            sb = pout.tile([P, 512], F32, tag="sb")
            nc.gpsimd.dma_start(
                out=sb,
                in_=bass.AP(tensor=rsc.tensor, offset=rsc.offset,
                            ap=[[0, P]] + [list(d) for d in rsc.ap]))
            nc.vector.reciprocal_approx_fast(out=rinvb[:, i0:i0 + 512],
                                             in_=sb)

        # PV: both channel tiles accumulate concurrently (8 banks), each
        # vT slice stationary across 4 matmuls
        psas = {(ct, h): ps_big.tile([P, 1024], F32,
                                     tag=["big", "pva"][ct],
                                     name=f"pv{ct}_{h}")
                for ct in range(CT) for h in range(2)}
        for jtp in range(JP):
            for ct in range(CT):
                vw = vT_dr[:, :, jtp, ct * P:(ct + 1) * P]
                for half in range(2):
                    for ii in range(2):
                        ib = 2 * half + ii
                        nc.tensor.matmul(
                            psas[(ct, half)][:, ii * 512:(ii + 1) * 512],
                            lhsT=vw,
                            rhs=eT[:, jtp, :, ib * 512:(ib + 1) * 512],
                            start=(jtp == 0), stop=(jtp == JP - 1),
                            perf_mode=DR)
        for ct in range(CT):
            for half in range(2):
                nc.scalar.activation(
                    out=A_sb[ct][:, half * 1024:(half + 1) * 1024],
                    in_=psas[(ct, half)], func=AF.Copy)

        # ---- output projection + normalization + bias + residual ----
        for dt in range(CT):
            for icp in range(2):
                i0 = icp * 1024
                ps = ps_big.tile([P, 1024], F32, tag=["big", "pva"][(dt + icp) % 2],
                                 name=f"pj{dt}_{icp}")
                for ii in range(2):
                    ic = 2 * icp + ii
                    for cc in range(CT):
                        nc.tensor.matmul(
                            ps[:, ii * 512:(ii + 1) * 512],
                            lhsT=w_sb[("wpT", cc)][:, dt * P:(dt + 1) * P],
                            rhs=A_sb[cc][:, ic * 512:(ic + 1) * 512],
                            start=(cc == 0), stop=(cc == CT - 1))
                ot = pout.tile([P, 1024], F32, tag="ot")
                nc.vector.tensor_mul(ot, ps, rinvb[:, i0:i0 + 1024])
                nc.vector.scalar_tensor_tensor(
                    out=ot, in0=ot, scalar=col_sb[("bpe", dt)],
                    in1=xh_sb[dt][:, i0:i0 + 1024],
                    op0=AluOpType.add, op1=AluOpType.add)
                nc.sync.dma_start(
                    out=out.ap()[dt * P:(dt + 1) * P, i0:i0 + 1024],
                    in_=ot)


def _get_program():
    if "nc" not in _cache:
        _cache["nc"] = _build_program()
    return _cache["nc"]


def kernel(x, gn_scale, gn_bias, wq, bq, wk, bk, wv, bv, wproj, bproj):
    x = np.asarray(x, dtype=np.float32)
    b, c, hh, ww = x.shape
    assert (b, c, hh * ww) == (B, C, N)
    xf = np.ascontiguousarray(x.reshape(B, C, N))

    bf = ml_dtypes.bfloat16
    # Channel-pair interleave permutation for DoubleRow: even channels then odd.
    perm = np.concatenate([np.arange(0, C, 2), np.arange(1, C, 2)])
    wqT_s = np.ascontiguousarray(np.asarray(wq, np.float32).T[:, perm]).astype(bf)
    bq_s = np.ascontiguousarray(np.asarray(bq, np.float32)[perm])
    wkT = np.ascontiguousarray(np.asarray(wk, np.float32).T[:, perm]).astype(bf)
    bk_s = np.ascontiguousarray(np.asarray(bk, np.float32)[perm])
    wvT = np.ascontiguousarray(np.asarray(wv, np.float32).T[:, perm]).astype(bf)
    wpT = np.ascontiguousarray(np.asarray(wproj, np.float32).T[perm, :]).astype(bf)
    # softmax rows sum to 1 => v-bias contributes wproj@bv, constant per channel
    bpe = (np.asarray(bproj, np.float64)
           + np.asarray(wproj, np.float64) @ np.asarray(bv, np.float64)
           ).astype(np.float32)

    g8 = np.zeros((P, P // GSIZE), np.float32)
    gt01 = np.zeros((P // GSIZE, P), np.float32)
    for ch in range(P):
        g8[ch, ch // GSIZE] = 1.0 / GSIZE   # yields per-group means directly
        gt01[ch // GSIZE, ch] = 1.0

    common = dict(gnsc=np.asarray(gn_scale, np.float32),
                  gnbs=np.asarray(gn_bias, np.float32),
                  g8=g8, gt01=gt01,
                  wqT=wqT_s, bq=bq_s, wkT=wkT, bk=bk_s,
                  wvT=wvT, wpT=wpT, bpe=bpe)

    in_maps = []
    for core in range(NCORES):
        bi, half = core // 2, core % 2
        in_maps.append(dict(
            x_full=np.ascontiguousarray(xf[bi]),
            xh=np.ascontiguousarray(xf[bi][:, half * NH:(half + 1) * NH]),
            **common))

    nc = _get_program()
    trace = bool(os.environ.get("BASS_KERNEL_TRACE"))
    res = run_bass_kernel_spmd(nc, in_maps, core_ids=list(range(NCORES)),
                               trace=trace)
    _cache["last_results"] = res

    full = np.empty((B, C, N), np.float32)
    for core in range(NCORES):
        bi, half = core // 2, core % 2
        full[bi][:, half * NH:(half + 1) * NH] = res.results[core]["out"]
    return full.reshape(B, C, hh, ww)


# revision 24
# speedup vs baseline: 1.1361x; 1.0237x over previous
# Trainium2 Bass kernel for NonLocalBlock (GroupNorm + 1x1-conv self-attention + residual).
#
# Full input x: [4, 256, 64, 64] f32. Output: x + proj(attn(gn(x))), same shape.
#
# Sharding: 8 cores = 4 batches x 2 query-halves. Attention is independent per
# batch; within a batch, softmax rows (queries) split cleanly across 2 cores.
# Each core redundantly computes GroupNorm + K + V^T for its batch (cheap), and
# computes scores/softmax/PV/proj only for its 2048 queries. No collectives.
#
# Per-core program (c = 256 channels as 2 partition-tiles, n = 4096 keys):
#   - GroupNorm stats: bn_stats/bn_aggr per channel, group-combine and
#     broadcast-back via tiny PE matmuls with 0/1 group matrices.
#   - h = x*A + B (bf16), plus the query half from a separate input slice so
#     all access patterns stay static across the SPMD program.
#   - k, q, vT in fp8-e4m3 with the contraction dim stored channel-interleaved
#     ([128, 2, *]), so the attention matmuls run in DoubleRow perf mode
#     (2 fp8 MACs/cell/cycle, K=256 per instruction). The interleave is
#     produced for free: host permutes weight columns; PSUM->SBUF copies land
#     each output-channel half in its pair plane.
#   - scores transposed: sT[j,i] = k^T q; exp on ACT fused with the
#     PSUM->SBUF copy (1/sqrt(c) folded into the activation scale); eT[j,i]
#     is then directly the PV moving operand - no transposes anywhere.
#   - row sums of exp via ones-vector DR matmuls; softmax normalization is a
#     column scaling that commutes through PV and proj, applied in the output
#     stage (reciprocal_approx_fast on a broadcast of the sums).
#   - bv never applied on-chip: softmax rows sum to 1, so wproj@bv folds into
#     bproj on the host. out = x_half + rinv * (wproj @ A_unnorm) + bproj_eff.
#
# Stationary-operand reuse: each k/vT slice serves all 4 query blocks
# back-to-back, so LDWEIGHTS is paid once per 4 matmuls.

import os
import sys

for _p in ("/opt/trn_rl_repo", "/root/.axon_site/_ro/trn_rl_repo"):
    if os.path.isdir(_p) and _p not in sys.path:
        sys.path.insert(0, _p)

import numpy as np
import ml_dtypes

import concourse.bass as bass
import concourse.tile as tile
from concourse import bacc, mybir
from concourse.alu_op_type import AluOpType
from concourse.bass_utils import run_bass_kernel_spmd

from concourse import dve_ops as _dvo
from concourse.dve_spec import Spec as _DveSpec, Src0 as _Src0, C0 as _C0, \
    C1 as _C1, C2 as _C2, sq as _sq, lower as _dve_lower
from concourse.dve_uop import DveOpSpec as _DveOpSpec
from concourse.dve_table_gen import dve_ver_for as _dve_ver_for


def _register_exp_q4():
    # out = (c2 + z*(c1 + z*c0))^4 ~= exp(z*s) when (c0,c1,c2) are the
    # quadratic Taylor of exp(z*s/4): lets the Vector engine carry half the
    # softmax exp load (single fused uop; ACT is otherwise the bottleneck).
    if "EXP_Q4_ANT" in _dvo._SUB_OPCODE_FOR_NAME:
        return next(op for op in _dvo.OPS if op.name == "EXP_Q4_ANT")
    ver = _dve_ver_for("TRN2")
    spec = _DveSpec(
        body=_sq(_sq(_C2 + _Src0 * (_C1 + _Src0 * _C0))),
        reference=lambda in0, in1, c0, c1, c2: (c2 + in0 * (c1 + in0 * c0)) ** 4)
    opcode = max(_dvo._SUB_OPCODE_FOR_NAME.values()) + 1
    sha = _DveOpSpec(name="EXP_Q4_ANT", opcode=opcode,
                     uops=_dve_lower(spec, ver=ver), rd1_en=False).sha(ver)
    op = _dvo.DveOp("EXP_Q4_ANT", spec, subdim=False, uops_sha={ver: sha})
    _dvo.OPS.append(op)
    _dvo.CUSTOM_DVE_SPECS["EXP_Q4_ANT"] = spec
    _dvo._SUB_OPCODE_FOR_NAME["EXP_Q4_ANT"] = opcode
    return op


EXP_Q4 = _register_exp_q4()

F32 = mybir.dt.float32
BF16 = mybir.dt.bfloat16
F8 = mybir.dt.float8e4
AF = mybir.ActivationFunctionType
DR = mybir.MatmulPerfMode.DoubleRow

B = 4
C = 256
N = 4096           # 64*64 spatial positions
NH = N // 2        # queries per core
GROUPS = 32
GSIZE = C // GROUPS  # 8 channels per group
EPS = 1e-6
P = 128
CT = C // P        # 2 channel tiles
JT = N // P        # 32 key tiles (16 DoubleRow pairs)
JP = JT // 2
NB = NH // 512     # 4 query blocks of 512
NCORES = 8
SCALE = float(1.0 / np.sqrt(C))

_cache = {}


def _col(ap_1d, ct):
    # View a [256] DRAM tensor as [256, 1] and take channel-tile ct's [128, 1].
    return ap_1d.ap().rearrange("(a b) -> a b", b=1)[ct * P:(ct + 1) * P, :]


def _build_program():
    nc = bacc.Bacc("TRN2", target_bir_lowering=False, debug=False)

    x_full = nc.dram_tensor("x_full", [C, N], BF16, kind="ExternalInput")
    xh = nc.dram_tensor("xh", [C, NH], F32, kind="ExternalInput")
    gnsc = nc.dram_tensor("gnsc", [C], F32, kind="ExternalInput")
    gnbs = nc.dram_tensor("gnbs", [C], F32, kind="ExternalInput")
    g8 = nc.dram_tensor("g8", [P, P // GSIZE], F32, kind="ExternalInput")
    gt01 = nc.dram_tensor("gt01", [P // GSIZE, P], F32, kind="ExternalInput")
    wqT = nc.dram_tensor("wqT", [C, C], BF16, kind="ExternalInput")
    bq = nc.dram_tensor("bq", [C], F32, kind="ExternalInput")
    wkT = nc.dram_tensor("wkT", [C, C], BF16, kind="ExternalInput")
    bk = nc.dram_tensor("bk", [C], F32, kind="ExternalInput")
    wvT = nc.dram_tensor("wvT", [C, C], BF16, kind="ExternalInput")
    wpT = nc.dram_tensor("wpT", [C, C], BF16, kind="ExternalInput")
    bpe = nc.dram_tensor("bpe", [C], F32, kind="ExternalInput")
    out = nc.dram_tensor("out", [C, NH], F32, kind="ExternalOutput")
    rinv_scr = nc.dram_tensor("rinv_scr", [NH], F32)

    with tile.TileContext(nc) as tc:
        _body(tc, x_full, xh, gnsc, gnbs, g8, gt01,
              wqT, bq, wkT, bk, wvT, wpT, bpe, out, rinv_scr)
    nc.compile()
    return nc


def _body(tc, x_full, xh, gnsc, gnbs, g8, gt01,
          wqT, bq, wkT, bk, wvT, wpT, bpe, out, rinv_scr):
    nc = tc.nc
    NG = P // GSIZE  # 16 groups per channel tile

    from contextlib import ExitStack
    with ExitStack() as ctx:
        consts = ctx.enter_context(tc.tile_pool(name="consts", bufs=1))
        px = ctx.enter_context(tc.tile_pool(name="px", bufs=1))
        ph = ctx.enter_context(tc.tile_pool(name="ph", bufs=1))
        pkv = ctx.enter_context(tc.tile_pool(name="pkv", bufs=1))
        pst = ctx.enter_context(tc.tile_pool(name="pst", bufs=4))
        pout = ctx.enter_context(tc.tile_pool(name="pout", bufs=3))
        # PSUM: two 2-bank score/misc slots + two 2-bank PV accumulators = 8
        ps_big = ctx.enter_context(tc.tile_pool(name="ps_big", bufs=2, space="PSUM"))
        ps_sum = ps_big

        # ---- x load first: 16 x 256KB pieces round-robin over all three
        # DMA queues, interleaved across channel tiles so bn_stats (one per
        # 512-column chunk) starts as each piece lands ----
        x_sb = [px.tile([P, N], BF16, tag=f"x{ct}", name=f"x{ct}")
                for ct in range(CT)]
        qs = [nc.sync, nc.scalar, nc.gpsimd]
        pi = 0
        for s in range(8):
            for ct in range(CT):
                qs[pi % 3].dma_start(
                    out=x_sb[ct][:, s * 512:(s + 1) * 512],
                    in_=x_full.ap()[ct * P:(ct + 1) * P, s * 512:(s + 1) * 512])
                pi += 1

        # ---- constants ----
        # DR weights need 16B-aligned pair-plane step; pad the ones vector
        ones8_t = consts.tile([P, 2, 16], F8, tag="ones")
        nc.vector.memset(ones8_t, 1.0)
        ones8 = ones8_t[:, :, 0:1]
        g8_sb = consts.tile([P, NG], F32, tag="g8")
        nc.sync.dma_start(out=g8_sb, in_=g8.ap())
        gt01_sb = consts.tile([NG, P], F32, tag="gt01")
        nc.scalar.dma_start(out=gt01_sb, in_=gt01.ap())

        w_sb = {}
        for name, h in (("wqT", wqT), ("wkT", wkT), ("wvT", wvT), ("wpT", wpT)):
            for ec in range(CT):
                t = consts.tile([P, C], BF16, tag=f"{name}{ec}")
                [nc.sync, nc.scalar][ec].dma_start(
                    out=t, in_=h.ap()[ec * P:(ec + 1) * P, :])
                w_sb[(name, ec)] = t

        col_sb = {}
        for name, h in (("gnsc", gnsc), ("gnbs", gnbs), ("bq", bq),
                        ("bk", bk), ("bpe", bpe)):
            for ct in range(CT):
                t = consts.tile([P, 1], F32, tag=f"{name}{ct}")
                [nc.sync, nc.scalar][ct].dma_start(out=t, in_=_col(h, ct))
                col_sb[(name, ct)] = t

        # ---- GroupNorm stats ----
        ab_cols = []
        for ct in range(CT):
            xt = x_sb[ct]
            stats = pst.tile([P, 8, nc.vector.BN_STATS_DIM], F32, tag="bnst")
            for s in range(8):
                nc.vector.bn_stats(out=stats[:, s, :], in_=xt[:, s * 512:(s + 1) * 512])
            mv = pst.tile([P, nc.vector.BN_AGGR_DIM], F32, tag="bnagg")
            nc.vector.bn_aggr(out=mv, in_=stats)

            # per-channel (mean, E[x^2]) -> per-group via G/8 matmul
            st2 = pst.tile([P, 2], F32, tag="st2")
            nc.vector.tensor_copy(out=st2[:, 0:1], in_=mv[:, 0:1])
            m2 = pst.tile([P, 1], F32, tag="m2")
            nc.vector.tensor_mul(m2, mv[:, 0:1], mv[:, 0:1])
            nc.vector.tensor_add(st2[:, 1:2], m2, mv[:, 1:2])

            gps = ps_big.tile([NG, 2], F32, tag="big")
            nc.tensor.matmul(gps, lhsT=g8_sb, rhs=st2, start=True, stop=True)
            gs = pst.tile([NG, 2], F32, tag="gs")
            nc.vector.tensor_copy(out=gs, in_=gps)

            # var_g = E[x^2]_g - mean_g^2 ; rstd = 1/sqrt(var+eps)
            vg = pst.tile([NG, 1], F32, tag="vg")
            nc.vector.tensor_mul(vg, gs[:, 0:1], gs[:, 0:1])
            nc.vector.tensor_tensor(out=vg, in0=gs[:, 1:2], in1=vg,
                                    op=AluOpType.subtract)
            eps_t = pst.tile([NG, 1], F32, tag="eps")
            nc.vector.memset(eps_t, EPS)
            std = pst.tile([NG, 1], F32, tag="std")
            nc.scalar.activation(out=std, in_=vg, func=AF.Sqrt, bias=eps_t, scale=1.0)
            rstd = pst.tile([NG, 1], F32, tag="rstd")
            nc.vector.reciprocal(out=rstd, in_=std)

            gs2 = pst.tile([NG, 2], F32, tag="gs2")
            nc.vector.tensor_copy(out=gs2[:, 0:1], in_=gs[:, 0:1])
            nc.vector.tensor_copy(out=gs2[:, 1:2], in_=rstd)

            # broadcast (mean_g, rstd_g) back to channels
            bps = ps_big.tile([P, 2], F32, tag="big")
            nc.tensor.matmul(bps, lhsT=gt01_sb, rhs=gs2, start=True, stop=True)
            mr = pst.tile([P, 2], F32, tag="mr")
            nc.vector.tensor_copy(out=mr, in_=bps)

            a_col = pst.tile([P, 1], F32, tag=f"acol{ct}")
            nc.vector.tensor_mul(a_col, mr[:, 1:2], col_sb[("gnsc", ct)])
            b_col = pst.tile([P, 1], F32, tag=f"bcol{ct}")
            nc.vector.tensor_mul(b_col, mr[:, 0:1], a_col)
            nc.vector.tensor_tensor(out=b_col, in0=col_sb[("gnbs", ct)],
                                    in1=b_col, op=AluOpType.subtract)
            ab_cols.append((a_col, b_col))

        # ---- h = x*A+B (chunked so k/vT matmuls start early); k, vT ----
        # k_sb/q_sb/vT_dr are fp8 with channels pair-interleaved for DoubleRow:
        # value (p, q, .) = channel 2p+q (host permuted the weight columns).
        h_sb = [ph.tile([P, N], BF16, tag=f"h{ct}", name=f"h{ct}") for ct in range(CT)]
        k_sb = pkv.tile([P, 2, N], F8, tag="k")
        vT_dr = pkv.tile([P, 2, JP, C], F8, tag="vT")
        for c4 in range(4):
            j0 = c4 * 1024
            for ct in range(CT):
                a_col, b_col = ab_cols[ct]
                nc.gpsimd.tensor_scalar(
                    out=h_sb[ct][:, j0:j0 + 1024], in0=x_sb[ct][:, j0:j0 + 1024],
                    scalar1=a_col, scalar2=b_col,
                    op0=AluOpType.mult, op1=AluOpType.add)
            for dt in range(CT):
                ps = ps_big.tile([P, 1024], F32, tag=["big", "pva"][(c4 + dt) % 2], name=f"k{c4}_{dt}")
                for jj in range(2):
                    jc = 2 * c4 + jj
                    for ec in range(CT):
                        nc.tensor.matmul(
                            ps[:, jj * 512:(jj + 1) * 512],
                            lhsT=w_sb[("wkT", ec)][:, dt * P:(dt + 1) * P],
                            rhs=h_sb[ec][:, jc * 512:(jc + 1) * 512],
                            start=(ec == 0), stop=(ec == CT - 1))
                nc.scalar.activation(
                    out=k_sb[:, dt, j0:j0 + 1024], in_=ps,
                    func=AF.Identity, bias=col_sb[("bk", dt)], scale=1.0)
            for t in (2 * c4, 2 * c4 + 1):
                # four jt per psum tile, quarters ordered (q, jtp) so one copy
                # lands them all in vT_dr[:, :, 2t:2t+2, :]
                ps = ps_big.tile([P, 4, C], F32, tag=["big", "pva"][t % 2],
                                 name=f"v{t}")
                for jj in range(4):
                    jt = 4 * t + jj
                    quarter = (jt % 2) * 2 + (jt // 2) % 2
                    for ec in range(CT):
                        nc.tensor.matmul(
                            ps[:, quarter, :],
                            lhsT=h_sb[ec][:, jt * P:(jt + 1) * P],
                            rhs=w_sb[("wvT", ec)],
                            start=(ec == 0), stop=(ec == CT - 1))
                nc.vector.tensor_copy(out=vT_dr[:, :, 2 * t:2 * t + 2, :],
                                      in_=ps)

        # ---- query-half h, q ----
        xh_sb, hh_sb = [], []
        for ct in range(CT):
            xht = px.tile([P, NH], F32, tag=f"x{ct}", name=f"xh{ct}")
            [nc.scalar, nc.sync][ct].dma_start(
                out=xht, in_=xh.ap()[ct * P:(ct + 1) * P, :])
            xh_sb.append(xht)
            a_col, b_col = ab_cols[ct]
            hht = ph.tile([P, NH], BF16, tag=f"hh{ct}", name=f"hh{ct}")
            nc.gpsimd.tensor_scalar(out=hht, in0=xht, scalar1=a_col, scalar2=b_col,
                                    op0=AluOpType.mult, op1=AluOpType.add)
            hh_sb.append(hht)

        q_sb = pkv.tile([P, 2, NH], F8, tag="q")
        for dt in range(CT):
            for icp in range(2):
                ps = ps_big.tile([P, 1024], F32, tag=["big", "pva"][(dt + icp) % 2], name=f"q{dt}_{icp}")
                for ii in range(2):
                    ic = 2 * icp + ii
                    for ec in range(CT):
                        nc.tensor.matmul(
                            ps[:, ii * 512:(ii + 1) * 512],
                            lhsT=w_sb[("wqT", ec)][:, dt * P:(dt + 1) * P],
                            rhs=hh_sb[ec][:, ic * 512:(ic + 1) * 512],
                            start=(ec == 0), stop=(ec == CT - 1))
                nc.scalar.activation(
                    out=q_sb[:, dt, icp * 1024:(icp + 1) * 1024], in_=ps,
                    func=AF.Identity, bias=col_sb[("bq", dt)], scale=1.0)

        # ---- attention: one pass over the 32 key tiles for all 2048
        # queries. Per key tile: 4 DR score matmuls; exp of the first query
        # half on ACT, of the second half on the Vector engine (fused q^4
        # polynomial - splitting exp across engines is what keeps PE fed).
        # PV for channel-tile 0 rides along; sums/PV-ct1/proj follow.
        # eT[p, jtp, q, i] = exp(s[j=(2*jtp+q)*128+p, i]/16)  (fp8)
        eT = pkv.tile([P, JP, 2, NH], F8, tag="eT")
        A_sb = [pkv.tile([P, NH], BF16, tag=f"A{ct}", name=f"A{ct}")
                for ct in range(CT)]
        rinvb = pkv.tile([P, NH], F32, tag="rinvb")
        EC1 = SCALE / 4.0
        EC0 = SCALE * SCALE / 32.0

        # scores: four rotating psum slots (both tag groups) so the two exp
        # engines pipeline freely; per jt, half0 exps on ACT, half1 on DVE
        for jt in range(JT):
            kw = k_sb[:, :, jt * P:(jt + 1) * P]
            for half in range(2):
                ps = ps_big.tile([P, 1024], F32,
                                 tag=["big", "pva"][jt % 2],
                                 name=f"sc{jt}_{half}")
                for ii in range(2):
                    ib = 2 * half + ii
                    nc.tensor.matmul(
                        ps[:, ii * 512:(ii + 1) * 512], lhsT=kw,
                        rhs=q_sb[:, :, ib * 512:(ib + 1) * 512],
                        start=True, stop=True, perf_mode=DR)
                dst = eT[:, jt // 2, jt % 2, half * 1024:(half + 1) * 1024]
                if half == 0:
                    nc.scalar.activation(out=dst, in_=ps, func=AF.Exp,
                                         scale=SCALE)
                else:
                    nc.vector._custom_dve(EXP_Q4, out=dst, in0=ps,
                                          s0=EC0, s1=EC1, imm2=1.0)

        # row sums: 4 accumulators spread over both tag groups
        pssums = [ps_big.tile([1, 512], F32, tag=["big", "pva"][ib % 2],
                              name=f"sm{ib}") for ib in range(NB)]
        for jtp in range(JP):
            for ib in range(NB):
                nc.tensor.matmul(pssums[ib], lhsT=ones8,
                                 rhs=eT[:, jtp, :, ib * 512:(ib + 1) * 512],
                                 start=(jtp == 0), stop=(jtp == JP - 1),
                                 perf_mode=DR)
        for ib in range(NB):
            i0 = ib * 512
            srow = pst.tile([1, 512], F32, tag="srow")
            nc.vector.tensor_copy(out=srow, in_=pssums[ib])
            nc.sync.dma_start(
                out=rinv_scr.ap().rearrange("(a b) -> a b", a=1)[:, i0:i0 + 512],
                in_=srow)
            rsc = rinv_scr.ap()[i0:i0 + 512]
            sb = pout.tile([P, 512], F32, tag="sb")
            nc.gpsimd.dma_start(
                out=sb,
                in_=bass.AP(tensor=rsc.tensor, offset=rsc.offset,
                            ap=[[0, P]] + [list(d) for d in rsc.ap]))
            nc.vector.reciprocal_approx_fast(out=rinvb[:, i0:i0 + 512],
                                             in_=sb)

        # PV: both channel tiles accumulate concurrently (8 banks), each
        # vT slice stationary across 4 matmuls
        psas = {(ct, h): ps_big.tile([P, 1024], F32,
                                     tag=["big", "pva"][ct],
                                     name=f"pv{ct}_{h}")
                for ct in range(CT) for h in range(2)}
        for jtp in range(JP):
            for ct in range(CT):
                vw = vT_dr[:, :, jtp, ct * P:(ct + 1) * P]
                for half in range(2):
                    for ii in range(2):
                        ib = 2 * half + ii
                        nc.tensor.matmul(
                            psas[(ct, half)][:, ii * 512:(ii + 1) * 512],
                            lhsT=vw,
                            rhs=eT[:, jtp, :, ib * 512:(ib + 1) * 512],
                            start=(jtp == 0), stop=(jtp == JP - 1),
                            perf_mode=DR)
        for ct in range(CT):
            for half in range(2):
                nc.scalar.activation(
                    out=A_sb[ct][:, half * 1024:(half + 1) * 1024],
                    in_=psas[(ct, half)], func=AF.Copy)

        # ---- output projection + normalization + bias + residual ----
        for dt in range(CT):
            for icp in range(2):
                i0 = icp * 1024
                ps = ps_big.tile([P, 1024], F32, tag=["big", "pva"][(dt + icp) % 2],
                                 name=f"pj{dt}_{icp}")
                for ii in range(2):
                    ic = 2 * icp + ii
                    for cc in range(CT):
                        nc.tensor.matmul(
                            ps[:, ii * 512:(ii + 1) * 512],
                            lhsT=w_sb[("wpT", cc)][:, dt * P:(dt + 1) * P],
                            rhs=A_sb[cc][:, ic * 512:(ic + 1) * 512],
                            start=(cc == 0), stop=(cc == CT - 1))
                ot = pout.tile([P, 1024], F32, tag="ot")
                nc.vector.tensor_mul(ot, ps, rinvb[:, i0:i0 + 1024])
                nc.vector.scalar_tensor_tensor(
                    out=ot, in0=ot, scalar=col_sb[("bpe", dt)],
                    in1=xh_sb[dt][:, i0:i0 + 1024],
                    op0=AluOpType.add, op1=AluOpType.add)
                nc.sync.dma_start(
                    out=out.ap()[dt * P:(dt + 1) * P, i0:i0 + 1024],
                    in_=ot)


def _get_program():
    if "nc" not in _cache:
        _cache["nc"] = _build_program()
    return _cache["nc"]


def kernel(x, gn_scale, gn_bias, wq, bq, wk, bk, wv, bv, wproj, bproj):
    x = np.asarray(x, dtype=np.float32)
    b, c, hh, ww = x.shape
    assert (b, c, hh * ww) == (B, C, N)
    xf = np.ascontiguousarray(x.reshape(B, C, N))

    bf = ml_dtypes.bfloat16
    # Channel-pair interleave permutation for DoubleRow: even channels then odd.
    perm = np.concatenate([np.arange(0, C, 2), np.arange(1, C, 2)])
    wqT_s = np.ascontiguousarray(np.asarray(wq, np.float32).T[:, perm]).astype(bf)
    bq_s = np.ascontiguousarray(np.asarray(bq, np.float32)[perm])
    wkT = np.ascontiguousarray(np.asarray(wk, np.float32).T[:, perm]).astype(bf)
    bk_s = np.ascontiguousarray(np.asarray(bk, np.float32)[perm])
    wvT = np.ascontiguousarray(np.asarray(wv, np.float32).T[:, perm]).astype(bf)
    wpT = np.ascontiguousarray(np.asarray(wproj, np.float32).T[perm, :]).astype(bf)
    # softmax rows sum to 1 => v-bias contributes wproj@bv, constant per channel
    bpe = (np.asarray(bproj, np.float64)
           + np.asarray(wproj, np.float64) @ np.asarray(bv, np.float64)
           ).astype(np.float32)

    g8 = np.zeros((P, P // GSIZE), np.float32)
    gt01 = np.zeros((P // GSIZE, P), np.float32)
    for ch in range(P):
        g8[ch, ch // GSIZE] = 1.0 / GSIZE   # yields per-group means directly
        gt01[ch // GSIZE, ch] = 1.0

    common = dict(gnsc=np.asarray(gn_scale, np.float32),
                  gnbs=np.asarray(gn_bias, np.float32),
                  g8=g8, gt01=gt01,
                  wqT=wqT_s, bq=bq_s, wkT=wkT, bk=bk_s,
                  wvT=wvT, wpT=wpT, bpe=bpe)

    in_maps = []
    for core in range(NCORES):
        bi, half = core // 2, core % 2
        in_maps.append(dict(
            x_full=np.ascontiguousarray(xf[bi]).astype(bf),
            xh=np.ascontiguousarray(xf[bi][:, half * NH:(half + 1) * NH]),
            **common))

    nc = _get_program()
    trace = bool(os.environ.get("BASS_KERNEL_TRACE"))
    res = run_bass_kernel_spmd(nc, in_maps, core_ids=list(range(NCORES)),
                               trace=trace)
    _cache["last_results"] = res

    full = np.empty((B, C, N), np.float32)
    for core in range(NCORES):
        bi, half = core // 2, core % 2
        full[bi][:, half * NH:(half + 1) * NH] = res.results[core]["out"]
    return full.reshape(B, C, hh, ww)
